# revision 65
# baseline (speedup 1.0000x reference)
"""Trainium2 Bass kernel for a deformable-DETR style decoder layer.

Strategy (8 NeuronCores): core c handles (batch b = c//2, head-group g = c%2,
heads [3g, 3g+3)).  The multi-scale deformable attention never materializes
the value projection of the whole src [S=66836, 384]; sampling locations are
computed on-device directly in a (corner-v x query) 128-partition layout
(v-dependent terms baked into host tables), corner-row pairs are fetched from
an fp8-e4m3, 512B-row-padded copy of src with 12 indirect-gather DMAs of at
most 1024 descriptors each (the SWDGE descriptor ring hangs beyond 1024 per
instruction), ordered per head [l4-B, levels 0-3 x2, l4-C] so each head's
combine and fused projection finish as soon as its tiles land; the level-4
windows carry complementary gates over int16 window indices.  Gathered rows
are combined with wide (attention x bilinear)
coefficient tiles via PE matmuls against a fixed 0/1 query mask, and only
then projected.  In the common all-trivial-bias case the per-head value and
output projections are folded into one host-precomputed [384, 384] weight
per head, accumulated into a single [Q, C] partial that a 2-rank AllReduce
completes; each core of a pair then runs the identical tail (LN, FFN, LN).

Latency-oriented choices: weights/biases/tables ship as a few packed DRAM
tensors loaded in strict per-ring use order; tgt/extra arrive pre-transposed
from the host so attention starts without PE transposes; attention keeps
scores transposed ([k, (h q)]) so PE does the softmax reductions (no
max-subtraction: scores are O(0.2) here); LN uses bn_stats moments and a
bitcast+Newton rsqrt so the Activation engine only ever needs the exp table;
the LNE normalization is folded into the offset/attention-weight projection
(transposes and matmuls run on the raw residual while the moments compute in
parallel; the affine correction is applied on the psum read-out); the index
wrap into dma_gather layout runs on PE (8 selection matmuls + 2 strided
int16 copies) followed by ~2.5us of dependency-pinned warm-up transposes
that hold the PE p-state at full clock into the combine burst; corner-weight
arithmetic runs on the Pool engine (comparisons must stay on DVE) and
psum->sbuf copies are spread over the Activation engine; all coefficient
tiles are emitted before any gather-dependent work so the in-order DVE queue
never stalls them; weight chunks load on the sync ring in strict use order
(the DMA engines are a FIFO, so issue order is priority); the FFN hidden
layer is produced directly in transposed chunks.

Known HW pitfall encoded here: a matmul whose lhsT/rhs sit at partition
offset 64 combined with a column-sliced PSUM output crashes the NEFF; all
matmuls therefore keep operands at base partition 0 (per-head column
layouts) when the output is a PSUM slice.

All per-core specialization (batch slice, head-sliced weights, sampling
tables) flows through the per-core input maps, so a single SPMD program runs
on all 8 cores.  Biases/LN params are checked on the host: the specialized
program (triv=True) is built when every bias except ms_off_b is zero and LN
gains are one (always true for this problem's setup_inputs); a general
fallback program is built otherwise.
"""
import numpy as np

B, Q, C, H, L, PTS, DFF, EXTRA = 4, 64, 384, 6, 5, 4, 1024, 128
SIZES = [(14, 14), (28, 28), (56, 56), (112, 112), (224, 224)]
S = sum(h * w for h, w in SIZES)
DH = C // H
N_CORES = 8
HG = 3                      # heads per core
NCOL = HG * L * PTS         # 60 sample columns per core
MAGIC = float(3 * 2 ** 22)  # 1.5*2^23 -> rne(x) for |x| < 2^22
JH = 5 * 4                  # itile columns per head (L*PTS)
WROW_B = 16660              # window B base row (start of level 4)
WROW_C = 49364              # window C base row
NW_B = WROW_C - WROW_B + 1  # 32705 rows in window B view
NW_C = 66836 - WROW_C - 1   # rows in window C view (max extent fits src)
CP = 512                    # fp8 src row stride (padded to 512B)
EPS = 1e-5
NIX = NCOL + 2 * HG * 4     # 84 index columns (A cols + l4 B + l4 C)

_CACHE = {}


# --- packed-constant layouts (shared by _emit / _build / make_in_maps) ---
# bf16 weight chunks: (chunk_name, ring, [(name, K, N), ...]); each weight
# wT [K, N] is stored as K//128 blocks of [128, N] side by side.
def wchunks(triv):
    ch = [
        ("wc_sa1", "sync", [("sa_v_wT", 384, 384)]),
        ("wc_sa1k", "sync", [("sa_k_wT", 384, 384)]),
        ("wc_sa2", "sync", [("sa_q_wT", 384, 384), ("sa_out_wT", 384, 384)]),
        ("wc_ea", "scalar", [("ea_vk_wT", 384, 768), ("ea_q_wT", 384, 384),
                             ("ea_out_wT", 384, 384)]),
    ]
    if triv:
        ch += [("wc_ms", "sync", [("offaw_wT", 384, 180), ("wf_wT", 384, 1152)]),
               ("wc_out", "scalar", [("ffn_w1T", 384, 1024)])]
    else:
        ch += [("wc_ms", "sync", [("offaw_wT", 384, 180), ("val_wT_g", 384, 192)]),
               ("wc_out", "scalar", [("out_wT", 384, 384), ("ffn_w1T", 384, 1024)])]
    ch += [("wc_ffn2", "scalar", [("ffn_w2T", 1024, 384)])]
    return ch


# bf16 bias row pack [1, NBIAS]
BIASROW = [("sa_in_b", 1152), ("sa_out_b", 384), ("ea_in_b", 1152),
           ("ea_out_b", 384), ("offaw_b", 180), ("val_b_g", 192),
           ("out_b", 384), ("ffn_b1", 1024), ("ffn_b2", 384)]
NBIAS = sum(n for _, n in BIASROW)
# f32 [Q, NQPK] pack: LN params (general fallback only)
QPK = [("ln2_g", C), ("ln2_b", C), ("lne_g", C),
       ("lne_b", C), ("ln1_g", C), ("ln1_b", C), ("ln3_g", C), ("ln3_b", C)]
NQPK = sum(n for _, n in QPK)
# f32 [128, NQPK2] pack: sampling tables in the (v, q) partition layout
# (p = v*64 + q; v-dependent values baked in per row)
QPK2 = [("xb2", NCOL), ("yb2", NCOL), ("wm12", NCOL), ("wm22", NCOL),
        ("hm22", NCOL), ("etab2", NCOL), ("wtab2", NCOL), ("base2", NCOL),
        ("refpts2", 2), ("sgn2", 1), ("vcol2", 1),
        ("wsum2", 3 * NCOL), ("ob2", 3 * NCOL)]
NQPK2 = sum(n for _, n in QPK2)


def _chunk_cols(items):
    return sum((k // 128) * n for _, k, n in items)


def _emit(tc, io, use_ag=True, triv=True, stop_after=None):
    """Emit the SPMD program for one core. io: dict name -> AP of dram tensors."""
    import concourse.bass as bass
    import concourse.mybir as mybir
    from concourse.masks import make_identity
    nc = tc.nc
    f32, bf16, i32 = mybir.dt.float32, mybir.dt.bfloat16, mybir.dt.int32
    f8 = mybir.dt.float8e4
    AL = mybir.AluOpType
    ACT = mybir.ActivationFunctionType
    AX = mybir.AxisListType

    from contextlib import ExitStack
    stack = ExitStack()
    wpool = stack.enter_context(tc.tile_pool(name="weights", bufs=1))
    sb = stack.enter_context(tc.tile_pool(name="work", bufs=1))
    ps = stack.enter_context(tc.tile_pool(name="psum", bufs=2, space="PSUM"))
    pse = ps
    dram = stack.enter_context(tc.tile_pool(name="dram", bufs=1, space="DRAM"))

    W = {}
    SMALL = {}
    BF_B = {}
    # activations + first-needed constants on the sync HWDGE ring, in use order
    xTp = wpool.tile([128, 3 * 64 + 3 * 128], bf16, name="xTp")
    nc.sync.dma_start(out=xTp[:], in_=io["xT_pack"][:])
    tgt0 = wpool.tile([Q, C], f32, name="tgt0")
    nc.sync.dma_start(out=tgt0[:], in_=io["tgt_in"][:])
    saT = [xTp[:, i * 64:(i + 1) * 64] for i in range(3)]
    eaT = [xTp[:, 192 + i * 128:192 + (i + 1) * 128] for i in range(3)]

    chunk_ring = {"sync": nc.sync, "scalar": nc.scalar, "pool": nc.gpsimd}
    WCH = wchunks(triv)

    def load_chunk(cname, ring=None, late=False):
        _, ring0, items = next(x for x in WCH if x[0] == cname)
        t = wpool.tile([128, _chunk_cols(items)], bf16, name=cname)
        eng = chunk_ring[ring or ring0]
        eng.dma_start(out=t[:], in_=io[cname][:])
        c0 = 0
        for name, k, n in items:
            W[name] = [t[:, c0 + i * n:c0 + (i + 1) * n] for i in range(k // 128)]
            c0 += (k // 128) * n

    # all early-phase loads on the sync ring in strict use order (the DMA
    # engines are a FIFO: a big low-priority chunk on another ring would
    # head-of-line-block wc_sa1); tail-phase weights go via the idle Pool
    # ring, LN params (fallback) via scalar
    load_chunk("wc_sa1")       # sync
    load_chunk("wc_sa1k")      # sync
    load_chunk("wc_sa2")       # sync
    load_chunk("wc_ea", ring="sync")
    biasrow = wpool.tile([1, NBIAS], bf16, name="biasrow")
    nc.sync.dma_start(out=biasrow[:], in_=io["biasrow"][:])
    c0 = 0
    for name, n in BIASROW:
        BF_B[name] = biasrow[0:1, c0:c0 + n]
        c0 += n
    if not triv:
        qpk = wpool.tile([Q, NQPK], f32, name="qpk")
        nc.scalar.dma_start(out=qpk[:], in_=io["qpk"][:])
        c0 = 0
        for name, n in QPK:
            SMALL[name] = qpk[:, c0:c0 + n]
            c0 += n
    load_chunk("wc_ms")        # sync
    qpk2 = wpool.tile([128, NQPK2], f32, name="qpk2")
    nc.sync.dma_start(out=qpk2[:], in_=io["qpk2"][:])
    c0 = 0
    for name, n in QPK2:
        SMALL[name] = qpk2[:, c0:c0 + n]
        c0 += n
    mpk = wpool.tile([128, Q + 9 * 128], f32, name="mpk")
    nc.sync.dma_start(out=mpk[:], in_=io["mpk"][:])
    mask_f = mpk[:, 0:Q]
    rmat = mpk[:, Q:Q + 8 * 128]
    maskT = mpk[0:Q, Q + 8 * 128:Q + 9 * 128]
    # tail-phase weights last on the sync ring: the DMA FIFO preserves issue
    # order, so these big chunks cannot head-of-line-block the early loads
    load_chunk("wc_out", ring="sync")
    load_chunk("wc_ffn2", ring="sync")
    mask_bf = wpool.tile([128, Q], bf16, name="mask_bf")
    nc.vector.tensor_copy(out=mask_bf[:], in_=mask_f)

    ident = wpool.tile([128, 128], f32, name="ident")
    make_identity(nc, ident[:])
    ident_bf = wpool.tile([128, 128], bf16, name="ident_bf")
    nc.vector.tensor_copy(out=ident_bf[:], in_=ident[:])
    zcol = wpool.tile([128, 1], f32, name="zcol")
    nc.vector.memset(zcol[:], 0.0)
    ones_bf = wpool.tile([1, 128], bf16, name="ones_bf")
    nc.vector.memset(ones_bf[:], 1.0)
    ones_bfc = wpool.tile([128, 1], bf16, name="ones_bfc")
    nc.vector.memset(ones_bfc[:], 1.0)
    rsq_magic = wpool.tile([128, 1], i32, name="rsq_magic")
    nc.vector.memset(rsq_magic[:], 0x5f3759df)

    def transpose_tiles(x, tag, dtype=None, ceng=None):
        """x: sbuf AP [p<=128, f] -> list of sbuf tiles [ck, p] (f in chunks of
        128). The transpose runs in x's dtype (f32 inputs allowed); the
        psum->sbuf copy converts to `dtype`."""
        dtype = dtype or f32
        in_f32 = x.dtype == f32
        idn = ident if in_f32 else ident_bf
        p, f = x.shape[0], x.shape[1]
        outs = []
        for i in range(0, f, 128):
            ck = min(128, f - i)
            pt = pse.tile([128, 128], f32 if in_f32 else bf16,
                          name=f"{tag}_tp_{i}", uniquify=True, tag="mm", bufs=2)
            nc.tensor.transpose(out=pt[:ck, :p], in_=x[:, i:i + ck], identity=idn[:p, :p])
            st = sb.tile([ck, p], dtype, name=f"{tag}_t_{i}", uniquify=True, tag=f"{tag}_t{i}")
            if ceng is nc.scalar:
                nc.scalar.copy(out=st[:], in_=pt[:ck, :p])
            else:
                nc.vector.tensor_copy(out=st[:], in_=pt[:ck, :p])
            outs.append(st)
        return outs

    def ln_add(x_res_a, x_res_b, tag):
        xs = sb.tile([Q, C], f32, name=f"{tag}_x", uniquify=True, tag=f"{tag}_x")
        nc.vector.tensor_tensor(out=xs[:], in0=x_res_a, in1=x_res_b, op=AL.add)
        return xs

    def ln_stats(xs, tag):
        """bn_stats moments + bit-trick/Newton rsqrt -> (mv [Q,2], rstd [Q,1])."""
        st6 = sb.tile([Q, 6], f32, name=f"{tag}_st", uniquify=True, tag=f"{tag}_st")
        nc.vector.bn_stats(out=st6[:], in_=xs[:])
        mv = sb.tile([Q, 2], f32, name=f"{tag}_mv", uniquify=True, tag=f"{tag}_mv")
        nc.vector.bn_aggr(out=mv[:], in_=st6[:])
        # rstd = rsqrt(var + eps) via the bit-trick seed + 2 Newton steps,
        # keeping the Activation engine (and its table loads) out of LN
        vv = sb.tile([Q, 1], f32, name=f"{tag}_vv", uniquify=True, tag=f"{tag}_vv")
        nc.vector.tensor_scalar(out=vv[:], in0=mv[:, 1:2], scalar1=EPS,
                                scalar2=None, op0=AL.add)
        rstd = sb.tile([Q, 1], f32, name=f"{tag}_rstd", uniquify=True, tag=f"{tag}_rstd")
        ri = rstd[:].bitcast(i32)
        nc.vector.tensor_scalar(out=ri, in0=vv[:].bitcast(i32), scalar1=1,
                                scalar2=None, op0=AL.logical_shift_right)
        nc.vector.tensor_tensor(out=ri, in0=rsq_magic[:Q, :], in1=ri,
                                op=AL.subtract)
        tn = sb.tile([Q, 1], f32, name=f"{tag}_tn", uniquify=True, tag=f"{tag}_tn")
        nc.vector.scalar_tensor_tensor(out=tn[:], in0=rstd[:], scalar=rstd[:, 0:1],
                                       in1=vv[:], op0=AL.mult, op1=AL.mult)
        nc.vector.tensor_scalar(out=tn[:], in0=tn[:], scalar1=-0.5, scalar2=1.5,
                                op0=AL.mult, op1=AL.add)
        nc.vector.tensor_tensor(out=rstd[:], in0=rstd[:], in1=tn[:], op=AL.mult)
        return mv, rstd

    def ln(x_res_a, x_res_b, gname, bname, tag):
        """LayerNorm(a + b) -> new sbuf tile [Q, C]."""
        xs = ln_add(x_res_a, x_res_b, tag)
        mv, rstd = ln_stats(xs, tag)
        nc.vector.tensor_scalar(out=xs[:], in0=xs[:], scalar1=mv[:, 0:1],
                                scalar2=rstd[:, 0:1], op0=AL.subtract, op1=AL.mult)
        if not triv:
            nc.vector.tensor_tensor(out=xs[:], in0=xs[:], in1=SMALL[gname], op=AL.mult)
            nc.vector.tensor_tensor(out=xs[:], in0=xs[:], in1=SMALL[bname], op=AL.add)
        return xs

    # ---------------- phase 1: self attention + extra cross attention -------
    def kv_proj(kvT, nk, wv, wk, b_in_bf, tag):
        """K/V projections from transposed input: v_sb [nk, C], kT [DH, H*nk].

        wvk: [384, 768] chunks, cols 0:C = V-out, C:2C = K-out.  kT is laid
        out per head at base partition 0 (HW rejects matmuls whose operands
        sit at partition offset 64 when the psum out is a column slice).  The
        k bias is dropped: softmax over k is invariant to it.
        """
        v_ps = ps.tile([nk, C], f32, name=f"{tag}_vps", uniquify=True,
                       tag="acc", bufs=1)
        for i in range(3):
            nc.tensor.matmul(v_ps[:], lhsT=kvT[i], rhs=wv[i][:, 0:C],
                             start=(i == 0), stop=(triv and i == 2))
        if not triv:
            nc.tensor.matmul(v_ps[:], lhsT=ones_bf[:1, :nk],
                             rhs=b_in_bf[:1, 2 * C:3 * C], start=False, stop=True)
        v_sb = sb.tile([nk, C], bf16, name=f"{tag}_v", uniquify=True, tag=f"{tag}_v")
        nc.scalar.copy(out=v_sb[:], in_=v_ps[:])
        kT_ps = pse.tile([DH, H * nk], f32, name=f"{tag}_kTp", uniquify=True,
                         tag="kq", bufs=1)
        for h in range(H):
            for i in range(3):
                nc.tensor.matmul(kT_ps[:, h * nk:(h + 1) * nk],
                                 lhsT=wk[i][:, h * DH:(h + 1) * DH],
                                 rhs=kvT[i], start=(i == 0), stop=(i == 2))
        kT_sb = sb.tile([DH, H * nk], bf16, name=f"{tag}_kT", uniquify=True,
                        tag=f"{tag}_kT")
        nc.scalar.copy(out=kT_sb[:], in_=kT_ps[:])
        return v_sb, kT_sb

    def attention(xT, kv, nk, wq, b_in_bf, woT, b_out_bf, tag):
        """MHA in bf16 (f32 psum), scores kept transposed [k, (h q)].

        No max-subtraction in the softmax (scores are O(0.2) here); the
        per-(q,h) normalizer is computed by PE column sums and applied to the
        unnormalized o. Returns out-proj psum [Q, C] f32."""
        v_sb, kT_sb = kv
        qT_ps = pse.tile([DH, H * Q], f32, name=f"{tag}_qTp", uniquify=True,
                         tag="kq", bufs=1)
        for h in range(H):
            for i in range(3):
                nc.tensor.matmul(qT_ps[:, h * Q:(h + 1) * Q],
                                 lhsT=wq[i][:, h * DH:(h + 1) * DH],
                                 rhs=xT[i], start=(i == 0),
                                 stop=(triv and i == 2))
            if not triv:
                nc.tensor.matmul(qT_ps[:, h * Q:(h + 1) * Q],
                                 lhsT=b_in_bf[:1, h * DH:(h + 1) * DH],
                                 rhs=ones_bf[:1, :Q], start=False, stop=True)
        qT_sb = sb.tile([DH, H * Q], bf16, name=f"{tag}_qT", uniquify=True,
                        tag=f"{tag}_qT")
        nc.scalar.copy(out=qT_sb[:], in_=qT_ps[:])
        # transposed scores for all heads in one psum strip [nk, H*Q]
        sT_ps = pse.tile([nk, H * Q], f32, name=f"{tag}_sT", uniquify=True,
                         tag="kq", bufs=1)
        for h in range(H):
            nc.tensor.matmul(sT_ps[:, h * Q:(h + 1) * Q],
                             lhsT=kT_sb[:, h * nk:(h + 1) * nk],
                             rhs=qT_sb[:, h * Q:(h + 1) * Q],
                             start=True, stop=True)
        eT = sb.tile([nk, H * Q], bf16, name=f"{tag}_eT", uniquify=True,
                     tag=f"{tag}_eT")
        nc.scalar.activation(out=eT[:], in_=sT_ps[:], func=ACT.Exp,
                             bias=zcol[:nk, :1], scale=1.0 / np.sqrt(DH))
        if stop_after == "exp":
            return None
        # per-(q,h) sums via PE column reduction; unnormalized o
        ssum_ps = pse.tile([Q, H], f32, name=f"{tag}_ssp", uniquify=True, tag="mm")
        for h in range(H):
            nc.tensor.matmul(ssum_ps[:, h:h + 1], lhsT=eT[:, h * Q:(h + 1) * Q],
                             rhs=ones_bfc[:nk, :1], start=True, stop=True)
        rinv = sb.tile([Q, H], f32, name=f"{tag}_ri", uniquify=True, tag=f"{tag}_ri")
        nc.vector.reciprocal(out=rinv[:], in_=ssum_ps[:])
        o_ps = ps.tile([Q, C], f32, name=f"{tag}_ops", uniquify=True, tag="acc", bufs=1)
        for h in range(H):
            nc.tensor.matmul(o_ps[:, h * DH:(h + 1) * DH],
                             lhsT=eT[:, h * Q:(h + 1) * Q],
                             rhs=v_sb[:, h * DH:(h + 1) * DH],
                             start=True, stop=True)
        # fused psum->sbuf copy + per-(q,h) normalize
        o_sb = sb.tile([Q, C], bf16, name=f"{tag}_o", uniquify=True, tag=f"{tag}_o")
        nc.vector.tensor_tensor(
            out=o_sb[:].rearrange("q (h d) -> q h d", h=H),
            in0=o_ps[:].rearrange("q (h d) -> q h d", h=H),
            in1=rinv[:].rearrange("q (h one) -> q h one", one=1
                                  ).broadcast_to([Q, H, DH]), op=AL.mult)
        if stop_after == "onorm":
            return None
        oT = transpose_tiles(o_sb[:], f"{tag}_o", dtype=bf16, ceng=nc.scalar)
        t2_ps = ps.tile([Q, C], f32, name=f"{tag}_t2", uniquify=True, tag="acc", bufs=1)
        for i in range(3):
            nc.tensor.matmul(t2_ps[:], lhsT=oT[i][:], rhs=woT[i][:, :],
                             start=(i == 0), stop=(triv and i == 2))
        if not triv:
            nc.tensor.matmul(t2_ps[:], lhsT=ones_bf[:1, :Q], rhs=b_out_bf[:1, :],
                             start=False, stop=True)
        return t2_ps

    if stop_after == "pre":
        nc.sync.dma_start(out=io["out"][:], in_=tgt0[:])
        stack.close()
        return
    sa_kv = kv_proj(saT, Q, W["sa_v_wT"], W["sa_k_wT"], BF_B["sa_in_b"], "sa")
    if stop_after == "kv":
        nc.sync.dma_start(out=io["out"][:], in_=tgt0[:])
        stack.close()
        return
    t2 = attention(saT, sa_kv, Q, W["sa_q_wT"], BF_B["sa_in_b"],
                   W["sa_out_wT"], BF_B["sa_out_b"], "sa")
    # EA K/V prep emitted after the SA attention matmuls: wc_ea lands ~8us,
    # and putting it earlier would head-of-line-block SA's PE stream
    ea_kv = kv_proj(eaT, EXTRA,
                    [w[:, 0:C] for w in W["ea_vk_wT"]],
                    [w[:, C:2 * C] for w in W["ea_vk_wT"]],
                    BF_B["ea_in_b"], "ea")
    if stop_after in ("exp", "onorm", "t2"):
        nc.sync.dma_start(out=io["out"][:], in_=tgt0[:])
        stack.close()
        return
    tgt1 = ln(tgt0[:], t2[:], "ln2_g", "ln2_b", "ln2")
    if stop_after == "sa":
        nc.sync.dma_start(out=io["out"][:], in_=tgt1[:])
        stack.close()
        return
    t1T = transpose_tiles(tgt1[:], "ea_x", dtype=bf16)
    t2 = attention([t[:] for t in t1T], ea_kv, EXTRA, W["ea_q_wT"],
                   BF_B["ea_in_b"], W["ea_out_wT"], BF_B["ea_out_b"], "ea")
    if triv:
        # LNE folded into the offset/aw projection: the transposes and
        # matmuls run on the raw residual x = tgt1 + t2 while the moment
        # statistics compute in parallel; the psum read-out applies
        # oa = rstd*(x@W) - (mu*rstd)*colsum(W) + b, which equals
        # (LN(x) @ W + b) exactly for unit gain / zero LN bias.
        xs2 = ln_add(tgt1[:], t2[:], "lne")
        src_t2 = xs2
    else:
        tgt2 = ln(tgt1[:], t2[:], "lne_g", "lne_b", "lne")
        src_t2 = tgt2
    if stop_after == "ea":
        nc.sync.dma_start(out=io["out"][:], in_=tgt1[:])
        stack.close()
        return

    # ---------------- phase 2: sampling locations ----------------
    # Everything below runs directly in the (p = v*64 + q) partition layout:
    # the y-slot (v) dependent values are baked into host tables (qpk2), so no
    # cross-partition shuffles are needed. x is row-duplicated into both
    # partition halves via a broadcast copy of its transposes.
    tgt2dT = []
    for i in range(3):
        pt = pse.tile([128, Q], f32, name=f"t2T_{i}", uniquify=True, tag="mm", bufs=2)
        nc.tensor.transpose(out=pt[:], in_=src_t2[:, i * 128:(i + 1) * 128],
                            identity=ident[:Q, :Q])
        st = sb.tile([128, 128], bf16, name=f"t2dT_{i}", uniquify=True, tag=f"t2dT{i}")
        nc.vector.tensor_copy(
            out=st[:].rearrange("p (u q) -> p u q", u=2),
            in_=pt[:, 0:Q].rearrange("p (one q) -> p one q", one=1
                                     ).broadcast_to([128, 2, Q]))
        tgt2dT.append(st)
    oa_ps = ps.tile([128, 3 * NCOL], f32, name="oa_ps", tag="acc", bufs=1)
    for i in range(3):
        nc.tensor.matmul(oa_ps[:], lhsT=tgt2dT[i][:], rhs=W["offaw_wT"][i][:, :],
                         start=(i == 0), stop=(triv and i == 2))
    oa = sb.tile([128, 3 * NCOL], f32, name="oa")
    if triv:
        mv2, rstd2 = ln_stats(xs2, "lne")
        mustd = sb.tile([Q, 2], f32, name="mustd")
        nc.vector.tensor_tensor(out=mustd[:, 0:1], in0=mv2[:, 0:1],
                                in1=rstd2[:], op=AL.mult)
        nc.vector.tensor_copy(out=mustd[:, 1:2], in_=rstd2[:])
        dup_ps = pse.tile([128, 2], f32, name="dup_ps", tag="mm")
        nc.tensor.matmul(dup_ps[:], lhsT=maskT, rhs=mustd[:], start=True,
                         stop=True)
        ms128 = sb.tile([128, 2], f32, name="ms128")
        nc.vector.tensor_copy(out=ms128[:], in_=dup_ps[:])
        nc.vector.tensor_scalar(out=oa[:], in0=oa_ps[:],
                                scalar1=ms128[:, 1:2], scalar2=None, op0=AL.mult)
        tmp180 = sb.tile([128, 3 * NCOL], f32, name="tmp180")
        nc.vector.scalar_tensor_tensor(out=tmp180[:], in0=SMALL["wsum2"],
                                       scalar=ms128[:, 0:1], in1=SMALL["ob2"],
                                       op0=AL.mult, op1=AL.subtract)
        nc.vector.tensor_tensor(out=oa[:], in0=oa[:], in1=tmp180[:],
                                op=AL.subtract)
        # materialize tgt2 = LN(x) for the later residual (off critical path)
        nc.vector.tensor_scalar(out=xs2[:], in0=xs2[:], scalar1=mv2[:, 0:1],
                                scalar2=rstd2[:, 0:1], op0=AL.subtract,
                                op1=AL.mult)
        tgt2 = xs2
    else:
        nc.tensor.matmul(oa_ps[:], lhsT=ones_bf[:1, :128],
                         rhs=BF_B["offaw_b"][:1, :], start=False, stop=True)
        nc.vector.tensor_copy(out=oa[:], in_=oa_ps[:])
    off4 = oa[:, :2 * NCOL].rearrange("p (c two) -> p c two", two=2)
    def t128(tag):
        return sb.tile([128, NCOL], f32, name=tag, uniquify=True, tag=tag)

    def floor_pair(val, tag, eng=None):
        """val is pre-shifted by (-0.5 + eps) via the host xb2/yb2 bases:
        rne(val) == floor of the true coordinate (exact except within eps of
        an integer, where bilinear continuity bounds the output error), and
        fr == frac - 0.5 + eps; consumers fold the constant back."""
        eng = eng or nc.vector
        fl = t128(f"{tag}_fl")
        eng.tensor_scalar(out=fl[:], in0=val, scalar1=MAGIC, scalar2=MAGIC,
                          op0=AL.add, op1=AL.subtract)
        fr = t128(f"{tag}_fr")
        eng.tensor_tensor(out=fr[:], in0=val, in1=fl[:], op=AL.subtract)
        return fl, fr

    # x side (identical in both partition halves): slot weights for u=0/1
    xx = t128("xx")
    nc.vector.tensor_tensor(out=xx[:], in0=SMALL["xb2"], in1=off4[:, :, 0],
                            op=AL.add)
    x0, fx = floor_pair(xx[:], "fx")
    xs = t128("xs")
    nc.vector.tensor_scalar(out=xs[:], in0=x0[:], scalar1=0.0, scalar2=None,
                            op0=AL.max)
    nc.vector.tensor_tensor(out=xs[:], in0=xs[:], in1=SMALL["wm22"], op=AL.min)
    # y side: v lives in the partition dim; single slot weight per partition
    yy = t128("yy")
    nc.vector.tensor_tensor(out=yy[:], in0=SMALL["yb2"], in1=off4[:, :, 1],
                            op=AL.add)
    y0, fy = floor_pair(yy[:], "fy")
    ys = t128("ys")
    nc.vector.tensor_scalar(out=ys[:], in0=y0[:], scalar1=0.0, scalar2=None,
                            op0=AL.max)
    nc.vector.tensor_tensor(out=ys[:], in0=ys[:], in1=SMALL["hm22"], op=AL.min)
    # row indices straight into the wrap source [128, 84]:
    # cols 0:60 = A rows (levels 0-3 used; l4 cols present but unused),
    # cols 60:72 = l4 B-window rows, 72:84 = l4 C-window rows
    idxsrc = sb.tile([128, NIX], f32, name="idxsrc")
    ifl = idxsrc[:, 0:NCOL]
    nc.vector.tensor_tensor(out=ifl, in0=ys[:], in1=SMALL["wtab2"], op=AL.mult)
    nc.vector.tensor_tensor(out=ifl, in0=ifl, in1=xs[:], op=AL.add)
    nc.vector.tensor_tensor(out=ifl, in0=ifl, in1=SMALL["base2"], op=AL.add)
    # A and B/C wrapped indices live in SEPARATE tiles: the A gathers'
    # descriptor generation must not wait (whole-tile dependency) on the
    # level-4 wrap copy, which can queue behind coefficient builds on DVE
    idx16a = [sb.tile([128, JH * 8], mybir.dt.int16, name=f"idx16a{h}",
                      uniquify=True, tag="i16a", bufs=3) for h in range(HG)]
    idx16bc = sb.tile([128, (NIX - NCOL) * 8], mybir.dt.int16, name="idx16bc")

    def wrap_cols(dst, c0, c1, tag):
        """Wrap idxsrc[:, c0:c1] into dst (int16, [128, (c1-c0)*8])."""
        n = c1 - c0
        dv = dst[:].rearrange("p (j g) -> p j g", g=8)
        for half in range(2):
            wr_ps = pse.tile([128, 4 * NIX], f32, name=f"wrp{tag}{half}",
                             uniquify=True, tag="mm")
            for gg in range(4):
                g = half * 4 + gg
                nc.tensor.matmul(wr_ps[:, gg * n:(gg + 1) * n],
                                 lhsT=rmat[:, g * 128:(g + 1) * 128],
                                 rhs=idxsrc[:, c0:c1], start=True, stop=True)
            nc.scalar.copy(
                out=dv[:, :, half * 4:(half + 1) * 4],
                in_=wr_ps[:, :4 * n].rearrange("p (g j) -> p j g", j=n))

    # per-head index tiles: head h's A gathers unblock as soon as its own
    # 20 columns are wrapped (whole-tile dependencies otherwise serialize
    # every gather behind the full wrap)
    for h in range(HG):
        wrap_cols(idx16a[h], h * JH, (h + 1) * JH, f"a{h}")

    # level-4 window split: B rows [16660, 49364), C rows [49364, 66836).
    # Every l4 (point, v) is gathered in both windows with complementary
    # gate coefficients; clipped rel indices keep reads in range.
    i4 = idxsrc[:, 0:NCOL].rearrange("p (h j) -> p h j", h=HG)[:, :, 16:20]
    iB = idxsrc[:, NCOL:NCOL + HG * 4].rearrange("p (h j) -> p h j", h=HG)
    nc.vector.tensor_scalar(out=iB, in0=i4, scalar1=-float(WROW_B),
                            scalar2=0.0, op0=AL.add, op1=AL.max)
    nc.vector.tensor_scalar(out=iB, in0=iB, scalar1=float(NW_B - 1), scalar2=None,
                            op0=AL.min)
    iC = idxsrc[:, NCOL + HG * 4:NIX].rearrange("p (h j) -> p h j", h=HG)
    nc.vector.tensor_scalar(out=iC, in0=i4, scalar1=-float(WROW_C),
                            scalar2=0.0, op0=AL.add, op1=AL.max)
    nc.vector.tensor_scalar(out=iC, in0=iC, scalar1=float(NW_C - 1), scalar2=None,
                            op0=AL.min)
    wrap_cols(idx16bc, NCOL, NIX, "bc")

    # Within a gather whose first idxsrc col is J0, index i = (j-J0)*128 + p
    # is read from idx16[i%16 (any replica row), j*8 + (i%128)//16].  The 8
    # selection matmuls R_g (R_g[p_in, p_out] = [p_in == g*16 + p_out%16])
    # produce out_g[p, j] = idxsrc[g*16 + p%16, j], i.e. the wrapped+replicated
    # layout directly in PSUM; two strided int16 copies interleave g into the
    # final column order.

    # PE warm-up: ~2.5us of dependency-pinned junk transposes right after the
    # wrap keeps the tensor engine's p-state ramped into the combine burst
    # (cold restarts cost 3.7x per matmul until 3us of continuous execution)
    for j in range(28):
        jp = pse.tile([128, 128], f32, name=f"warm{j}", uniquify=True,
                      tag="mm", bufs=2)
        nc.tensor.transpose(out=jp[:NIX, :128], in_=idxsrc[:, 0:NIX],
                            identity=ident[:, :])

    # unnormalized attention weights: exp only; 1/sum applied on the psum
    # read-out of each head's combined sum
    eaw = sb.tile([128, NCOL], f32, name="eaw")
    nc.scalar.activation(out=eaw[:], in_=oa[:, 2 * NCOL:3 * NCOL], func=ACT.Exp,
                         bias=zcol[:128, :1], scale=1.0)
    awsum = sb.tile([128, HG], f32, name="awsum")
    nc.vector.reduce_sum(
        out=awsum[:].rearrange("p (h one) -> p h one", one=1),
        in_=eaw[:].rearrange("p (h j) -> p h j", h=HG), axis=AX.X)
    rinv_aw = sb.tile([128, HG], f32, name="rinv_aw")
    nc.vector.reciprocal(out=rinv_aw[:], in_=awsum[:])

    # corner-weight chains: comparisons must stay on DVE (the Pool ISA
    # rejects is_ge/is_le/is_equal); the mult/add arithmetic runs on the
    # otherwise idle Pool engine in parallel with DVE
    gex = t128("gex")
    nc.vector.tensor_scalar(out=gex[:], in0=x0[:], scalar1=0.0, scalar2=None,
                            op0=AL.is_ge)
    lex = t128("lex")
    nc.vector.tensor_tensor(out=lex[:], in0=x0[:], in1=SMALL["wm22"], op=AL.is_le)
    em1x = t128("em1x")
    nc.vector.tensor_scalar(out=em1x[:], in0=x0[:], scalar1=-1.0, scalar2=None,
                            op0=AL.is_equal)
    eWx = t128("eWx")
    nc.vector.tensor_tensor(out=eWx[:], in0=x0[:], in1=SMALL["wm12"], op=AL.is_equal)
    gey = t128("gey")
    nc.vector.tensor_scalar(out=gey[:], in0=y0[:], scalar1=0.0, scalar2=None,
                            op0=AL.is_ge)
    ley = t128("ley")
    nc.vector.tensor_tensor(out=ley[:], in0=y0[:], in1=SMALL["hm22"], op=AL.is_le)
    edgy = t128("edgy")
    nc.vector.tensor_tensor(out=edgy[:], in0=y0[:], in1=SMALL["etab2"],
                            op=AL.is_equal)
    veng = nc.gpsimd
    inbx = t128("inbx")
    veng.tensor_tensor(out=inbx[:], in0=gex[:], in1=lex[:], op=AL.mult)
    omfx = t128("omfx")
    veng.tensor_scalar(out=omfx[:], in0=fx[:], scalar1=-1.0,
                       scalar2=0.5 + 2.0 ** -9, op0=AL.mult, op1=AL.add)
    wx0 = t128("wx0")
    veng.tensor_tensor(out=wx0[:], in0=omfx[:], in1=inbx[:], op=AL.mult)
    tmpx = t128("tmpx")
    nc.vector.scalar_tensor_tensor(out=tmpx[:], in0=fx[:],
                                   scalar=0.5 - 2.0 ** -9, in1=em1x[:],
                                   op0=AL.add, op1=AL.mult)
    veng.tensor_tensor(out=wx0[:], in0=wx0[:], in1=tmpx[:], op=AL.add)
    wx1 = t128("wx1")
    nc.vector.scalar_tensor_tensor(out=wx1[:], in0=fx[:],
                                   scalar=0.5 - 2.0 ** -9, in1=inbx[:],
                                   op0=AL.add, op1=AL.mult)
    veng.tensor_tensor(out=tmpx[:], in0=omfx[:], in1=eWx[:], op=AL.mult)
    veng.tensor_tensor(out=wx1[:], in0=wx1[:], in1=tmpx[:], op=AL.add)
    inby = t128("inby")
    veng.tensor_tensor(out=inby[:], in0=gey[:], in1=ley[:], op=AL.mult)
    # alpha = fy*sgn + vcol  (v=0 -> 1-fy, v=1 -> fy); wy = a*inb + (1-a)*edge
    alp = t128("alp")
    veng.tensor_scalar(out=alp[:], in0=fy[:], scalar1=SMALL["sgn2"][:, 0:1],
                            scalar2=SMALL["vcol2"][:, 0:1], op0=AL.mult, op1=AL.add)
    bet = t128("bet")
    veng.tensor_scalar(out=bet[:], in0=alp[:], scalar1=-1.0, scalar2=1.0,
                            op0=AL.mult, op1=AL.add)
    wy = t128("wy")
    veng.tensor_tensor(out=wy[:], in0=alp[:], in1=inby[:], op=AL.mult)
    veng.tensor_tensor(out=bet[:], in0=bet[:], in1=edgy[:], op=AL.mult)
    veng.tensor_tensor(out=wy[:], in0=wy[:], in1=bet[:], op=AL.add)

    # coefficients ctile[p, (h j u)] = wy * wx_u * eaw (aw-unnormalized)
    ctile = sb.tile([128, 2 * NCOL], f32, name="ctile")
    ct3 = ctile[:].rearrange("p (j u) -> p j u", u=2)
    cdup = t128("cdup")
    nc.vector.tensor_tensor(out=cdup[:], in0=wy[:], in1=eaw[:], op=AL.mult)
    nc.vector.tensor_tensor(out=ct3[:, :, 0], in0=cdup[:], in1=wx0[:], op=AL.mult)
    nc.vector.tensor_tensor(out=ct3[:, :, 1], in0=cdup[:], in1=wx1[:], op=AL.mult)

    gB4 = sb.tile([128, HG * 4], f32, name="gB4")
    nc.vector.tensor_scalar(out=gB4[:].rearrange("p (h j) -> p h j", h=HG),
                            in0=i4, scalar1=float(WROW_C), scalar2=None,
                            op0=AL.is_lt)
    # gated l4 coefficients
    c_l4 = ctile[:].rearrange("p (h j u) -> p h j u", h=HG, u=2)[:, :, 16:20, :]
    gB4b = gB4[:].rearrange("p (h j one) -> p h j one", h=HG, one=1
                            ).broadcast_to([128, HG, 4, 2])
    ctileB = sb.tile([128, HG * 8], f32, name="ctileB")
    ctileC = sb.tile([128, HG * 8], f32, name="ctileC")
    cB4 = ctileB[:].rearrange("p (h j u) -> p h j u", h=HG, u=2)
    cC4 = ctileC[:].rearrange("p (h j u) -> p h j u", h=HG, u=2)
    nc.vector.tensor_tensor(out=cB4, in0=c_l4, in1=gB4b, op=AL.mult)
    nc.vector.tensor_tensor(out=cC4, in0=c_l4, in1=cB4, op=AL.subtract)

    if stop_after == "phase2":
        dbg = dram.tile([128, JH * 8], mybir.dt.int16, name="dbg16")
        nc.sync.dma_start(out=dbg[:], in_=idx16a[0][:])
        nc.sync.dma_start(out=io["out"][:], in_=tgt2[:])
        stack.close()
        return

    # ---------------- phase 3: gather + combine + per-head projection ----------
    # 5 merged gathers (small B/C windows first so per-head combines can
    # finish — and per-head projections start — as soon as A_h lands):
    #   B_all/C_all: all heads' level-4 window tiles, 1536 idx each
    #   A_h:         per-head levels 0-3, 2048 idx
    # every gather stays <= 1024 descriptors (the SWDGE ring hangs on HW
    # beyond that); A tiles are filled by two 1024-descriptor gathers into
    # halves of one buffer
    def gather(out_view, base, nwin, c0, ncols):
        if c0 < NCOL:
            it, cb = idx16a[c0 // JH], c0 % JH
        else:
            it, cb = idx16bc, c0 - NCOL
        nc.gpsimd.dma_gather(
            out_ap=out_view,
            in_ap=bass.AP(io["srcflat_f8"].tensor, base * CP,
                          [[CP, nwin], [1, 2 * CP]]),
            idxs_ap=it[:, cb * 8:(cb + ncols) * 8],
            num_idxs=ncols * 128, num_idxs_reg=ncols * 128,
            elem_size=2 * CP, elem_step=CP)

    # per-head gather order [A_h x2, B_h, C_h]: head h's combine and fused
    # projection can run while head h+1 still streams
    gA, gBh, gCh = [], [], []
    for h in range(HG):
        ga = sb.tile([128, 16 * 2 * CP], f8, name=f"gA{h}", uniquify=True,
                     tag="gA", bufs=3)
        gav = ga[:].rearrange("p (b e) -> p b e", e=2 * CP)
        for k in range(2):
            gather(gav[:, k * 8:(k + 1) * 8, :], 0, WROW_B, h * JH + k * 8, 8)
        gA.append(ga)
        gb = sb.tile([128, 4 * 2 * CP], f8, name=f"gB{h}", uniquify=True,
                     tag="gBC", bufs=6)
        gather(gb[:].rearrange("p (b e) -> p b e", e=2 * CP),
               WROW_B, NW_B, NCOL + h * 4, 4)
        gBh.append(gb)
        gc = sb.tile([128, 4 * 2 * CP], f8, name=f"gC{h}", uniquify=True,
                     tag="gBC", bufs=6)
        gather(gc[:].rearrange("p (b e) -> p b e", e=2 * CP),
               WROW_C, NW_C, NCOL + HG * 4 + h * 4, 4)
        gCh.append(gc)

    # wide coefficient tiles CT[p, (k q)] = mask[p, q] * coef[p, k] built in
    # one DVE op per gather tile (bf16 for 2x DVE throughput)
    def ct_wide(coef_sl, ncols, tag):
        t = sb.tile([128, ncols * Q], bf16, name=tag, uniquify=True, tag=tag)
        nc.vector.tensor_tensor(
            out=t[:].rearrange("p (k q) -> p k q", q=Q),
            in0=mask_bf[:].rearrange("p (one q) -> p one q", one=1
                                     ).broadcast_to([128, ncols, Q]),
            in1=coef_sl.rearrange("p (k one) -> p k one", one=1
                                  ).broadcast_to([128, ncols, Q]),
            op=AL.mult)
        return t

    # all coefficient tiles are emitted before any combine/tail work so the
    # in-order DVE queue never stalls a later head's coefficients behind an
    # earlier head's gather-dependent tail ops
    ctA = [ct_wide(ctile[:, h * 2 * JH:h * 2 * JH + 32], 32, f"ctA{h}")
           for h in range(HG)]
    ctB = [ct_wide(ctileB[:, h * 8:(h + 1) * 8], 8, f"ctB{h}")
           for h in range(HG)]
    ctC = [ct_wide(ctileC[:, h * 8:(h + 1) * 8], 8, f"ctC{h}")
           for h in range(HG)]

    oh_ps = [ps.tile([Q, C], f32, name=f"oh_ps{h}", uniquify=True, tag="oh",
                     bufs=3) for h in range(HG)]
    nmm = [0] * HG
    NMM = 2 * 16 + 2 * 4 + 2 * 4

    def combine(h, gt, nb, bbase, ct):
        """oh_ps[h] += sum over nb (b,u) units from gather tile gt."""
        g3 = gt[:].rearrange("p (b e) -> p b e", e=2 * CP)
        for b in range(nb):
            for u in range(2):
                k = 2 * b + u
                nc.tensor.matmul(oh_ps[h][:], lhsT=ct[:, k * Q:(k + 1) * Q],
                                 rhs=g3[:, bbase + b, u * CP:u * CP + C],
                                 start=(nmm[h] == 0), stop=(nmm[h] == NMM - 1))
                nmm[h] += 1

    if triv:
        # per-head: normalize on psum read-out, transpose, and accumulate the
        # fused (val @ out) projection into one [Q, C] partial
        mo_ps = ps.tile([Q, C], f32, name="mo_ps", tag="acc", bufs=1)
        first_mo = [True]

        def head_tail(h):
            oh_sb = sb.tile([Q, C], bf16, name=f"oh_sb{h}", uniquify=True,
                            tag="oh_sb")
            nc.vector.tensor_scalar(out=oh_sb[:], in0=oh_ps[h][:],
                                    scalar1=rinv_aw[0:Q, h:h + 1], scalar2=None,
                                    op0=AL.mult)
            ohT = transpose_tiles(oh_sb[:], f"ohT{h}", dtype=bf16,
                                  ceng=nc.scalar)
            for i in range(3):
                nc.tensor.matmul(mo_ps[:], lhsT=ohT[i][:],
                                 rhs=W["wf_wT"][i][:, h * C:(h + 1) * C],
                                 start=(first_mo[0] and i == 0),
                                 stop=(h == HG - 1 and i == 2))
            first_mo[0] = False

        # head h's tail is emitted after head h+1's combine: the in-order PE
        # queue then overlaps the oh_sb/transpose roundtrip with real matmuls
        for h in range(HG):
            combine(h, gA[h], 16, 0, ctA[h])
            combine(h, gBh[h], 4, 0, ctB[h])
            combine(h, gCh[h], 4, 0, ctC[h])
            if h >= 1:
                head_tail(h - 1)
        head_tail(HG - 1)
        mo_sb = sb.tile([Q, C], f32, name="mo_sb")
        nc.vector.tensor_copy(out=mo_sb[:], in_=mo_ps[:])
        # ------- phase 4: 2-rank AllReduce of the partial output -------
        mo_full = sb.tile([Q, C], f32, name="mo_full")
        if use_ag:
            cc_in = dram.tile([Q, C], f32, name="cc_in")
            cc_out = dram.tile([Q, C], f32, name="cc_out")
            nc.gpsimd.dma_start(out=cc_in[:], in_=mo_sb[:])
            nc.gpsimd.collective_compute(
                "AllReduce", mybir.AluOpType.add,
                replica_groups=[[0, 1], [2, 3], [4, 5], [6, 7]],
                ins=[cc_in[:].opt()], outs=[cc_out[:].opt()])
            nc.sync.dma_start(out=mo_full[:], in_=cc_out[:])
        else:
            mo_full = mo_sb
    else:
        heads_sb = sb.tile([Q, HG * DH], f32, name="heads_sb")
        for h in range(HG):
            combine(h, gA[h], 16, 0, ctA[h])
            combine(h, gBh[h], 4, 0, ctB[h])
            combine(h, gCh[h], 4, 0, ctC[h])
            # sum of coefficients (for value-bias correction): swT [1, Q]
            red = sb.tile([128, 1], f32, name=f"red{h}", uniquify=True, tag="red")
            nc.vector.reduce_sum(out=red[:],
                                 in_=ctile[:, h * 2 * JH:(h + 1) * 2 * JH],
                                 axis=AX.X)
            nc.vector.tensor_tensor(out=red[:], in0=red[:],
                                    in1=rinv_aw[:, h:h + 1], op=AL.mult)
            swT_ps = pse.tile([1, Q], f32, name=f"swTp{h}", uniquify=True, tag="mm")
            nc.tensor.matmul(swT_ps[:], lhsT=red[:], rhs=mask_f, start=True,
                             stop=True)
            swT = sb.tile([1, Q], bf16, name=f"swT{h}", uniquify=True, tag="swT")
            nc.vector.tensor_copy(out=swT[:], in_=swT_ps[:])
            oh_sb = sb.tile([Q, C], bf16, name=f"oh_sb{h}", uniquify=True,
                            tag="oh_sb")
            nc.vector.tensor_scalar(out=oh_sb[:], in0=oh_ps[h][:],
                                    scalar1=rinv_aw[0:Q, h:h + 1], scalar2=None,
                                    op0=AL.mult)
            ohT = transpose_tiles(oh_sb[:], f"ohT{h}", dtype=bf16)
            pj_ps = pse.tile([Q, DH], f32, name=f"pj{h}", uniquify=True, tag="mm")
            for i in range(3):
                nc.tensor.matmul(pj_ps[:], lhsT=ohT[i][:],
                                 rhs=W["val_wT_g"][i][:, h * DH:(h + 1) * DH],
                                 start=(i == 0), stop=False)
            nc.tensor.matmul(pj_ps[:], lhsT=swT[:1, :],
                             rhs=BF_B["val_b_g"][:1, h * DH:(h + 1) * DH],
                             start=False, stop=True)
            nc.vector.tensor_copy(out=heads_sb[:, h * DH:(h + 1) * DH],
                                  in_=pj_ps[:])
        # ------- phase 4: exchange head groups (2-rank AllGather) -------
        headsfull = sb.tile([Q, C], f32, name="headsfull")
        if use_ag:
            cc_in = dram.tile([Q, HG * DH], f32, name="cc_in")
            cc_out = dram.tile([2 * Q, HG * DH], f32, name="cc_out")
            nc.gpsimd.dma_start(out=cc_in[:], in_=heads_sb[:])
            nc.gpsimd.collective_compute(
                "AllGather", mybir.AluOpType.bypass,
                replica_groups=[[0, 1], [2, 3], [4, 5], [6, 7]],
                ins=[cc_in[:].opt()], outs=[cc_out[:].opt()])
            nc.sync.dma_start(out=headsfull[:, 0:HG * DH], in_=cc_out[0:Q, :])
            nc.sync.dma_start(out=headsfull[:, HG * DH:C], in_=cc_out[Q:2 * Q, :])
        else:
            nc.vector.tensor_copy(out=headsfull[:, 0:HG * DH], in_=heads_sb[:])
            nc.vector.tensor_copy(out=headsfull[:, HG * DH:C], in_=heads_sb[:])
        hfT = transpose_tiles(headsfull[:], "hfT", dtype=bf16)
        mo_ps = ps.tile([Q, C], f32, name="mo_ps", tag="acc", bufs=1)
        for i in range(3):
            nc.tensor.matmul(mo_ps[:], lhsT=hfT[i][:], rhs=W["out_wT"][i][:, :],
                             start=(i == 0), stop=False)
        nc.tensor.matmul(mo_ps[:], lhsT=ones_bf[:1, :Q], rhs=BF_B["out_b"][:1, :],
                         start=False, stop=True)
        mo_full = mo_ps

    # ---------------- phase 5: LN + FFN + LN ----------------
    tgt3 = ln(tgt2[:], mo_full[:], "ln1_g", "ln1_b", "ln1")

    tgt3T = transpose_tiles(tgt3[:], "t3T", dtype=bf16)
    # h1 computed directly in transposed layout (chunks of 128 DFF rows) so
    # no [Q, DFF] transpose stage is needed before the second matmul
    h1T = []
    for m in range(8):
        f1_ps = pse.tile([128, Q], f32, name=f"f1_{m}", uniquify=True, tag="mm")
        for i in range(3):
            nc.tensor.matmul(f1_ps[:], lhsT=W["ffn_w1T"][i][:, m * 128:(m + 1) * 128],
                             rhs=tgt3T[i][:], start=(i == 0),
                             stop=(triv and i == 2))
        if not triv:
            nc.tensor.matmul(f1_ps[:], lhsT=BF_B["ffn_b1"][:1, m * 128:(m + 1) * 128],
                             rhs=ones_bf[:1, :Q], start=False, stop=True)
        t = sb.tile([128, Q], bf16, name=f"h1T_{m}", uniquify=True, tag="h1T",
                    bufs=3)
        if m % 2 == 0:
            nc.scalar.activation(out=t[:], in_=f1_ps[:], func=ACT.Relu,
                                 bias=zcol[:128, :1])
        else:
            nc.vector.tensor_scalar(out=t[:], in0=f1_ps[:], scalar1=0.0,
                                    scalar2=None, op0=AL.max)
        h1T.append(t)
    f2_ps = ps.tile([Q, C], f32, name="f2_ps", tag="acc", bufs=1)
    for i in range(8):
        nc.tensor.matmul(f2_ps[:], lhsT=h1T[i][:], rhs=W["ffn_w2T"][i][:, :],
                         start=(i == 0), stop=(triv and i == 7))
    if not triv:
        nc.tensor.matmul(f2_ps[:], lhsT=ones_bf[:1, :Q], rhs=BF_B["ffn_b2"][:1, :],
                         start=False, stop=True)
    out_sb = ln(tgt3[:], f2_ps[:], "ln3_g", "ln3_b", "ln3")
    nc.sync.dma_start(out=io["out"][:], in_=out_sb[:])
    stack.close()


def _build(n_devices=N_CORES, use_ag=True, triv=True, loop=1, stop_after=None):
    import concourse.bacc as bacc
    import concourse.mybir as mybir
    import concourse.tile as tile
    from concourse._compat import axon_active
    f32 = mybir.dt.float32
    nc = bacc.Bacc("TRN2", target_bir_lowering=False, debug=not axon_active(),
                   num_devices=n_devices)
    io = {}
    for name, shape in [("tgt_in", [Q, C]),
                        ("mpk", [128, Q + 9 * 128]),
                        ("qpk2", [128, NQPK2])]:
        io[name] = nc.dram_tensor(name, shape, f32, kind="ExternalInput").ap()
    if not triv:
        io["qpk"] = nc.dram_tensor("qpk", [Q, NQPK], f32, kind="ExternalInput").ap()
    io["xT_pack"] = nc.dram_tensor("xT_pack", [128, 3 * 64 + 3 * 128],
                                   mybir.dt.bfloat16, kind="ExternalInput").ap()
    io["srcflat_f8"] = nc.dram_tensor("srcflat_f8", [S, CP], mybir.dt.float8e4,
                                      kind="ExternalInput").ap()
    io["biasrow"] = nc.dram_tensor("biasrow", [1, NBIAS], mybir.dt.bfloat16,
                                   kind="ExternalInput").ap()
    for cname, _, items in wchunks(triv):
        io[cname] = nc.dram_tensor(cname, [128, _chunk_cols(items)],
                                   mybir.dt.bfloat16, kind="ExternalInput").ap()
    io["out"] = nc.dram_tensor("out", [Q, C], f32, kind="ExternalOutput").ap()

    with tile.TileContext(nc) as tc:
        for _ in range(loop):
            _emit(tc, io, use_ag=use_ag, triv=triv, stop_after=stop_after)
    nc.compile()
    return nc


def _is_triv(inp):
    zeros = ["sa_in_b", "sa_out_b", "ea_in_b", "ea_out_b", "ms_val_b",
             "ms_out_b", "ffn_b1", "ffn_b2", "ln2_b", "lne_b", "ln1_b", "ln3_b"]
    ones = ["ln2_g", "lne_g", "ln1_g", "ln3_g"]
    return (all(not np.any(np.asarray(inp[n])) for n in zeros)
            and all(np.all(np.asarray(inp[n]) == 1.0) for n in ones))


def make_in_maps(inputs, triv):
    """Build the 8 per-core input maps from the full problem inputs (numpy)."""
    import ml_dtypes
    inp = {k: np.ascontiguousarray(np.asarray(v, dtype=np.float32))
           if not k.startswith("src_") or k == "src" else np.asarray(v)
           for k, v in inputs.items()}
    lsi = np.asarray(inputs["src_level_start_index"]).astype(np.int64)
    spat = np.asarray(inputs["src_spatial_shapes"]).astype(np.int64)
    Wl = spat[:, 1].astype(np.float32)
    Hl = spat[:, 0].astype(np.float32)
    lcol = np.tile(np.repeat(np.arange(L), PTS), HG)  # [NCOL]
    mask = np.zeros((128, Q), np.float32)
    mask[np.arange(128), np.arange(128) % Q] = 1.0

    def wT(a):
        return np.ascontiguousarray(a.T.astype(np.float32))

    def repl(a):
        return np.ascontiguousarray(
            np.broadcast_to(a.reshape(1, -1), (Q, C)).astype(np.float32))

    # R_g[p_in, p_out] = 1 iff p_in == g*16 + p_out%16 (PE wrap matrices)
    pin = np.arange(128)[:, None]
    pout = np.arange(128)[None, :]
    mpk = np.zeros((128, Q + 9 * 128), np.float32)
    mpk[:, :Q] = mask
    for g in range(8):
        mpk[:, Q + g * 128:Q + (g + 1) * 128] = (
            pin == g * 16 + pout % 16).astype(np.float32)
    mpk[:Q, Q + 8 * 128:] = mask.T  # [Q, 128] partition-duplication matrix

    def pack_chunk(items, warrs):
        """warrs: name -> wT array [K, N]; returns [128, cols] bf16."""
        blocks = []
        for name, k, n in items:
            a = warrs[name]
            assert a.shape == (k, n), (name, a.shape, (k, n))
            for i in range(k // 128):
                blocks.append(a[i * 128:(i + 1) * 128])
        return np.ascontiguousarray(
            np.concatenate(blocks, axis=1).astype(ml_dtypes.bfloat16))

    # fp8 src with rows padded to CP bytes (exact 2-row gather descriptors)
    srcpads = []
    for b in range(B):
        sp = np.zeros((S, CP), ml_dtypes.float8_e4m3)
        sp[:, :C] = np.asarray(inp["src"][b]).astype(ml_dtypes.float8_e4m3)
        srcpads.append(sp)

    sa_wT = wT(inp["sa_in_w"])
    ea_wT = wT(inp["ea_in_w"])
    WCH = wchunks(triv)

    in_maps = []
    for c in range(N_CORES):
        b, g = c // 2, c % 2
        heads = range(HG * g, HG * g + HG)
        vr = np.asarray(inp["src_valid_ratios"])[b]  # [L, 2]
        off_rows = np.concatenate([np.arange(h * L * PTS * 2, (h + 1) * L * PTS * 2)
                                   for h in heads])
        aw_rows = np.concatenate([np.arange(h * L * PTS, (h + 1) * L * PTS)
                                  for h in heads])
        offaw_w = np.concatenate([inp["ms_off_w"][off_rows],
                                  inp["ms_attn_w"][aw_rows]], axis=0)  # [180, C]
        offaw_b = np.concatenate([inp["ms_off_b"][off_rows],
                                  inp["ms_attn_b"][aw_rows]])
        vcols = np.concatenate([np.arange(h * DH, (h + 1) * DH) for h in heads])
        warrs = dict(
            sa_v_wT=np.ascontiguousarray(sa_wT[:, 2 * C:3 * C]),
            sa_k_wT=np.ascontiguousarray(sa_wT[:, C:2 * C]),
            sa_q_wT=np.ascontiguousarray(sa_wT[:, 0:C]),
            sa_out_wT=wT(inp["sa_out_w"]),
            ea_vk_wT=np.concatenate([ea_wT[:, 2 * C:3 * C], ea_wT[:, C:2 * C]],
                                    axis=1),
            ea_q_wT=np.ascontiguousarray(ea_wT[:, 0:C]),
            ea_out_wT=wT(inp["ea_out_w"]),
            offaw_wT=wT(offaw_w), ffn_w1T=wT(inp["ffn_w1"]),
            ffn_w2T=wT(inp["ffn_w2"]))
        if triv:
            # fused per-head (value @ out) projections: [384, 384] per head
            wf = [(inp["ms_out_w"][:, h * DH:(h + 1) * DH]
                   @ inp["ms_val_w"][h * DH:(h + 1) * DH, :]).T
                  for h in heads]
            warrs["wf_wT"] = np.ascontiguousarray(
                np.concatenate(wf, axis=1).astype(np.float32))
        else:
            warrs["val_wT_g"] = wT(inp["ms_val_w"][vcols])
            warrs["out_wT"] = wT(inp["ms_out_w"])
        biases = dict(
            sa_in_b=inp["sa_in_b"], sa_out_b=inp["sa_out_b"],
            ea_in_b=inp["ea_in_b"], ea_out_b=inp["ea_out_b"],
            offaw_b=offaw_b, val_b_g=inp["ms_val_b"][vcols],
            out_b=inp["ms_out_b"], ffn_b1=inp["ffn_b1"], ffn_b2=inp["ffn_b2"])
        biasrow = np.concatenate([biases[name].reshape(-1) for name, _ in BIASROW])
        # host-transposed activations: saT [128, 3*64] + eaT [128, 3*128]
        tgtb = inp["tgt"][b]
        extb = inp["extra_memory"][b]
        xT_pack = np.concatenate(
            [tgtb[:, i * 128:(i + 1) * 128].T for i in range(3)]
            + [extb[:, i * 128:(i + 1) * 128].T for i in range(3)], axis=1)
        # (v, q)-partition tables: row p = v*64 + q
        vcol = np.repeat(np.arange(2), Q).astype(np.float32)[:, None]  # [128,1]
        rep2 = lambda row: np.ascontiguousarray(np.broadcast_to(
            row[None, :], (128, NCOL)).astype(np.float32))
        q2parts = dict(
            xb2=(np.tile(np.asarray(inp["reference_points"][b], np.float32),
                         (2, 1))[:, 0:1] * rep2(vr[lcol, 0] * Wl[lcol])
                 - 1.0 + 2.0 ** -9).astype(np.float32),
            yb2=(np.tile(np.asarray(inp["reference_points"][b], np.float32),
                         (2, 1))[:, 1:2] * rep2(vr[lcol, 1] * Hl[lcol])
                 - 1.0 + 2.0 ** -9).astype(np.float32),
            wm12=rep2(Wl[lcol] - 1), wm22=rep2(Wl[lcol] - 2),
            hm22=rep2(Hl[lcol] - 2),
            # y-edge: v=0 -> y0 == -1; v=1 -> y0 == H-1
            etab2=np.where(vcol > 0, rep2(Hl[lcol] - 1), -1.0).astype(np.float32),
            wtab2=rep2(Wl[lcol]),
            base2=(rep2(lsi[lcol].astype(np.float32))
                   + vcol * rep2(Wl[lcol])).astype(np.float32),
            refpts2=np.tile(np.asarray(inp["reference_points"][b], np.float32),
                            (2, 1)),
            sgn2=(2.0 * vcol - 1.0).astype(np.float32),
            vcol2=((1.0 - vcol) + (0.5 - 2.0 ** -9) * (2.0 * vcol - 1.0)
                   ).astype(np.float32),
            wsum2=np.ascontiguousarray(np.broadcast_to(
                offaw_w.sum(1)[None, :], (128, 3 * NCOL)).astype(np.float32)),
            ob2=np.ascontiguousarray(np.broadcast_to(
                offaw_b[None, :], (128, 3 * NCOL)).astype(np.float32)))
        qpk2 = np.concatenate([q2parts[name].reshape(128, n)
                               for name, n in QPK2], axis=1)
        m = dict(
            tgt_in=np.ascontiguousarray(tgtb),
            xT_pack=np.ascontiguousarray(xT_pack.astype(ml_dtypes.bfloat16)),
            srcflat_f8=srcpads[b],
            biasrow=np.ascontiguousarray(
                biasrow.reshape(1, -1).astype(ml_dtypes.bfloat16)),
            mpk=mpk,
            qpk2=np.ascontiguousarray(qpk2.astype(np.float32)),
        )
        if not triv:
            qparts = dict(
                ln2_g=repl(inp["ln2_g"]), ln2_b=repl(inp["ln2_b"]),
                lne_g=repl(inp["lne_g"]), lne_b=repl(inp["lne_b"]),
                ln1_g=repl(inp["ln1_g"]), ln1_b=repl(inp["ln1_b"]),
                ln3_g=repl(inp["ln3_g"]), ln3_b=repl(inp["ln3_b"]))
            m["qpk"] = np.ascontiguousarray(np.concatenate(
                [qparts[name].reshape(Q, n) for name, n in QPK],
                axis=1).astype(np.float32))
        for cname, _, items in WCH:
            m[cname] = pack_chunk(items, warrs)
        in_maps.append(m)
    return in_maps


def kernel(**inputs):
    import os
    from concourse.bass_utils import run_bass_kernel_spmd
    triv = _is_triv(inputs) and os.environ.get("KERNEL_FORCE_TRIV") != "0"
    key = ("nc", triv)
    if key not in _CACHE:
        _CACHE[key] = _build(triv=triv)
    nc = _CACHE[key]
    in_maps = make_in_maps(inputs, triv)
    res = run_bass_kernel_spmd(nc, in_maps, core_ids=list(range(N_CORES)))
    out = np.zeros((B, Q, C), np.float32)
    for b in range(B):
        out[b] = res.results[2 * b]["out"]
    return out


# revision 66
# speedup vs baseline: 1.0006x; 1.0006x over previous
"""Trainium2 Bass kernel for a deformable-DETR style decoder layer.

Strategy (8 NeuronCores): core c handles (batch b = c//2, head-group g = c%2,
heads [3g, 3g+3)).  The multi-scale deformable attention never materializes
the value projection of the whole src [S=66836, 384]; sampling locations are
computed on-device directly in a (corner-v x query) 128-partition layout
(v-dependent terms baked into host tables), corner-row pairs are fetched from
an fp8-e4m3, 512B-row-padded copy of src with 12 indirect-gather DMAs of at
most 1024 descriptors each (the SWDGE descriptor ring hangs beyond 1024 per
instruction), ordered per head [l4-B, levels 0-3 x2, l4-C] so each head's
combine and fused projection finish as soon as its tiles land; the level-4
windows carry complementary gates over int16 window indices.  Gathered rows
are combined with wide (attention x bilinear)
coefficient tiles via PE matmuls against a fixed 0/1 query mask, and only
then projected.  In the common all-trivial-bias case the per-head value and
output projections are folded into one host-precomputed [384, 384] weight
per head, accumulated into a single [Q, C] partial that a 2-rank AllReduce
completes; each core of a pair then runs the identical tail (LN, FFN, LN).

Latency-oriented choices: weights/biases/tables ship as a few packed DRAM
tensors loaded in strict per-ring use order; tgt/extra arrive pre-transposed
from the host so attention starts without PE transposes; attention keeps
scores transposed ([k, (h q)]) so PE does the softmax reductions (no
max-subtraction: scores are O(0.2) here); LN uses bn_stats moments and a
bitcast+Newton rsqrt so the Activation engine only ever needs the exp table;
the LNE normalization is folded into the offset/attention-weight projection
(transposes and matmuls run on the raw residual while the moments compute in
parallel; the affine correction is applied on the psum read-out); the index
wrap into dma_gather layout runs on PE (8 selection matmuls + 2 strided
int16 copies) followed by ~2.5us of dependency-pinned warm-up transposes
that hold the PE p-state at full clock into the combine burst; corner-weight
arithmetic runs on the Pool engine (comparisons must stay on DVE) and
psum->sbuf copies are spread over the Activation engine; all coefficient
tiles are emitted before any gather-dependent work so the in-order DVE queue
never stalls them; weight chunks load on the sync ring in strict use order
(the DMA engines are a FIFO, so issue order is priority); the FFN hidden
layer is produced directly in transposed chunks.

Known HW pitfall encoded here: a matmul whose lhsT/rhs sit at partition
offset 64 combined with a column-sliced PSUM output crashes the NEFF; all
matmuls therefore keep operands at base partition 0 (per-head column
layouts) when the output is a PSUM slice.

All per-core specialization (batch slice, head-sliced weights, sampling
tables) flows through the per-core input maps, so a single SPMD program runs
on all 8 cores.  Biases/LN params are checked on the host: the specialized
program (triv=True) is built when every bias except ms_off_b is zero and LN
gains are one (always true for this problem's setup_inputs); a general
fallback program is built otherwise.
"""
import numpy as np

B, Q, C, H, L, PTS, DFF, EXTRA = 4, 64, 384, 6, 5, 4, 1024, 128
SIZES = [(14, 14), (28, 28), (56, 56), (112, 112), (224, 224)]
S = sum(h * w for h, w in SIZES)
DH = C // H
N_CORES = 8
HG = 3                      # heads per core
NCOL = HG * L * PTS         # 60 sample columns per core
MAGIC = float(3 * 2 ** 22)  # 1.5*2^23 -> rne(x) for |x| < 2^22
JH = 5 * 4                  # itile columns per head (L*PTS)
WROW_B = 16660              # window B base row (start of level 4)
WROW_C = 49364              # window C base row
NW_B = WROW_C - WROW_B + 1  # 32705 rows in window B view
NW_C = 66836 - WROW_C - 1   # rows in window C view (max extent fits src)
CP = 512                    # fp8 src row stride (padded to 512B)
EPS = 1e-5
NIX = NCOL + 2 * HG * 4     # 84 index columns (A cols + l4 B + l4 C)

_CACHE = {}


# --- packed-constant layouts (shared by _emit / _build / make_in_maps) ---
# bf16 weight chunks: (chunk_name, ring, [(name, K, N), ...]); each weight
# wT [K, N] is stored as K//128 blocks of [128, N] side by side.
def wchunks(triv):
    ch = [
        ("wc_sa1", "sync", [("xT_packW", 128, 576), ("sa_v_wT", 384, 384)]),
        ("wc_sa1k", "sync", [("sa_k_wT", 384, 384)]),
        ("wc_sa2", "sync", [("sa_q_wT", 384, 384), ("sa_out_wT", 384, 384)]),
        ("wc_ea", "scalar", [("ea_vk_wT", 384, 768), ("ea_q_wT", 384, 384),
                             ("ea_out_wT", 384, 384)]),
    ]
    if triv:
        ch += [("wc_ms", "sync", [("offaw_wT", 384, 180), ("wf_wT", 384, 1152)]),
               ("wc_out", "scalar", [("ffn_w1T", 384, 1024)])]
    else:
        ch += [("wc_ms", "sync", [("offaw_wT", 384, 180), ("val_wT_g", 384, 192)]),
               ("wc_out", "scalar", [("out_wT", 384, 384), ("ffn_w1T", 384, 1024)])]
    ch += [("wc_ffn2", "scalar", [("ffn_w2T", 1024, 384)])]
    return ch


# bf16 bias row pack [1, NBIAS]
BIASROW = [("sa_in_b", 1152), ("sa_out_b", 384), ("ea_in_b", 1152),
           ("ea_out_b", 384), ("offaw_b", 180), ("val_b_g", 192),
           ("out_b", 384), ("ffn_b1", 1024), ("ffn_b2", 384)]
NBIAS = sum(n for _, n in BIASROW)
# f32 [Q, NQPK] pack: LN params (general fallback only)
QPK = [("ln2_g", C), ("ln2_b", C), ("lne_g", C),
       ("lne_b", C), ("ln1_g", C), ("ln1_b", C), ("ln3_g", C), ("ln3_b", C)]
NQPK = sum(n for _, n in QPK)
# f32 [128, NQPK2] pack: sampling tables in the (v, q) partition layout
# (p = v*64 + q; v-dependent values baked in per row)
QPK2 = [("xb2", NCOL), ("yb2", NCOL), ("wm12", NCOL), ("wm22", NCOL),
        ("hm22", NCOL), ("etab2", NCOL), ("wtab2", NCOL), ("base2", NCOL),
        ("refpts2", 2), ("sgn2", 1), ("vcol2", 1),
        ("wsum2", 3 * NCOL), ("ob2", 3 * NCOL)]
NQPK2 = sum(n for _, n in QPK2)


def _chunk_cols(items):
    return sum((k // 128) * n for _, k, n in items)


def _emit(tc, io, use_ag=True, triv=True, stop_after=None):
    """Emit the SPMD program for one core. io: dict name -> AP of dram tensors."""
    import concourse.bass as bass
    import concourse.mybir as mybir
    from concourse.masks import make_identity
    nc = tc.nc
    f32, bf16, i32 = mybir.dt.float32, mybir.dt.bfloat16, mybir.dt.int32
    f8 = mybir.dt.float8e4
    AL = mybir.AluOpType
    ACT = mybir.ActivationFunctionType
    AX = mybir.AxisListType

    from contextlib import ExitStack
    stack = ExitStack()
    wpool = stack.enter_context(tc.tile_pool(name="weights", bufs=1))
    sb = stack.enter_context(tc.tile_pool(name="work", bufs=1))
    ps = stack.enter_context(tc.tile_pool(name="psum", bufs=2, space="PSUM"))
    pse = ps
    dram = stack.enter_context(tc.tile_pool(name="dram", bufs=1, space="DRAM"))

    W = {}
    SMALL = {}
    BF_B = {}
    chunk_ring = {"sync": nc.sync, "scalar": nc.scalar, "pool": nc.gpsimd}
    WCH = wchunks(triv)

    def load_chunk(cname, ring=None, late=False):
        _, ring0, items = next(x for x in WCH if x[0] == cname)
        t = wpool.tile([128, _chunk_cols(items)], bf16, name=cname)
        eng = chunk_ring[ring or ring0]
        eng.dma_start(out=t[:], in_=io[cname][:])
        c0 = 0
        for name, k, n in items:
            W[name] = [t[:, c0 + i * n:c0 + (i + 1) * n] for i in range(k // 128)]
            c0 += (k // 128) * n

    # all early-phase loads on the sync ring in strict use order (the DMA
    # engines are a FIFO: a big low-priority chunk on another ring would
    # head-of-line-block wc_sa1); tail-phase weights go via the idle Pool
    # ring, LN params (fallback) via scalar
    # wc_sa1 carries the host-transposed activations too: one DMA + one
    # completion semaphore instead of two gates the first K/V matmuls
    load_chunk("wc_sa1")       # sync
    xTp = W["xT_packW"][0]
    saT = [xTp[:, i * 64:(i + 1) * 64] for i in range(3)]
    eaT = [xTp[:, 192 + i * 128:192 + (i + 1) * 128] for i in range(3)]
    load_chunk("wc_sa1k")      # sync
    load_chunk("wc_sa2")       # sync
    tgt0 = wpool.tile([Q, C], f32, name="tgt0")
    nc.sync.dma_start(out=tgt0[:], in_=io["tgt_in"][:])
    load_chunk("wc_ea", ring="sync")
    biasrow = wpool.tile([1, NBIAS], bf16, name="biasrow")
    nc.sync.dma_start(out=biasrow[:], in_=io["biasrow"][:])
    c0 = 0
    for name, n in BIASROW:
        BF_B[name] = biasrow[0:1, c0:c0 + n]
        c0 += n
    if not triv:
        qpk = wpool.tile([Q, NQPK], f32, name="qpk")
        nc.scalar.dma_start(out=qpk[:], in_=io["qpk"][:])
        c0 = 0
        for name, n in QPK:
            SMALL[name] = qpk[:, c0:c0 + n]
            c0 += n
    load_chunk("wc_ms")        # sync
    qpk2 = wpool.tile([128, NQPK2], f32, name="qpk2")
    nc.sync.dma_start(out=qpk2[:], in_=io["qpk2"][:])
    c0 = 0
    for name, n in QPK2:
        SMALL[name] = qpk2[:, c0:c0 + n]
        c0 += n
    mpk = wpool.tile([128, Q + 9 * 128], f32, name="mpk")
    nc.sync.dma_start(out=mpk[:], in_=io["mpk"][:])
    mask_f = mpk[:, 0:Q]
    rmat = mpk[:, Q:Q + 8 * 128]
    maskT = mpk[0:Q, Q + 8 * 128:Q + 9 * 128]
    # tail-phase weights last on the sync ring: the DMA FIFO preserves issue
    # order, so these big chunks cannot head-of-line-block the early loads
    load_chunk("wc_out", ring="sync")
    load_chunk("wc_ffn2", ring="sync")
    mask_bf = wpool.tile([128, Q], bf16, name="mask_bf")
    nc.vector.tensor_copy(out=mask_bf[:], in_=mask_f)

    ident = wpool.tile([128, 128], f32, name="ident")
    make_identity(nc, ident[:])
    ident_bf = wpool.tile([128, 128], bf16, name="ident_bf")
    nc.vector.tensor_copy(out=ident_bf[:], in_=ident[:])
    zcol = wpool.tile([128, 1], f32, name="zcol")
    nc.vector.memset(zcol[:], 0.0)
    ones_bf = wpool.tile([1, 128], bf16, name="ones_bf")
    nc.vector.memset(ones_bf[:], 1.0)
    ones_bfc = wpool.tile([128, 1], bf16, name="ones_bfc")
    nc.vector.memset(ones_bfc[:], 1.0)
    rsq_magic = wpool.tile([128, 1], i32, name="rsq_magic")
    nc.vector.memset(rsq_magic[:], 0x5f3759df)

    def transpose_tiles(x, tag, dtype=None, ceng=None):
        """x: sbuf AP [p<=128, f] -> list of sbuf tiles [ck, p] (f in chunks of
        128). The transpose runs in x's dtype (f32 inputs allowed); the
        psum->sbuf copy converts to `dtype`."""
        dtype = dtype or f32
        in_f32 = x.dtype == f32
        idn = ident if in_f32 else ident_bf
        p, f = x.shape[0], x.shape[1]
        outs = []
        for i in range(0, f, 128):
            ck = min(128, f - i)
            pt = pse.tile([128, 128], f32 if in_f32 else bf16,
                          name=f"{tag}_tp_{i}", uniquify=True, tag="mm", bufs=2)
            nc.tensor.transpose(out=pt[:ck, :p], in_=x[:, i:i + ck], identity=idn[:p, :p])
            st = sb.tile([ck, p], dtype, name=f"{tag}_t_{i}", uniquify=True, tag=f"{tag}_t{i}")
            if ceng is nc.scalar:
                nc.scalar.copy(out=st[:], in_=pt[:ck, :p])
            else:
                nc.vector.tensor_copy(out=st[:], in_=pt[:ck, :p])
            outs.append(st)
        return outs

    def ln_add(x_res_a, x_res_b, tag):
        xs = sb.tile([Q, C], f32, name=f"{tag}_x", uniquify=True, tag=f"{tag}_x")
        nc.vector.tensor_tensor(out=xs[:], in0=x_res_a, in1=x_res_b, op=AL.add)
        return xs

    def ln_stats(xs, tag):
        """bn_stats moments + bit-trick/Newton rsqrt -> (mv [Q,2], rstd [Q,1])."""
        st6 = sb.tile([Q, 6], f32, name=f"{tag}_st", uniquify=True, tag=f"{tag}_st")
        nc.vector.bn_stats(out=st6[:], in_=xs[:])
        mv = sb.tile([Q, 2], f32, name=f"{tag}_mv", uniquify=True, tag=f"{tag}_mv")
        nc.vector.bn_aggr(out=mv[:], in_=st6[:])
        # rstd = rsqrt(var + eps) via the bit-trick seed + 2 Newton steps,
        # keeping the Activation engine (and its table loads) out of LN
        vv = sb.tile([Q, 1], f32, name=f"{tag}_vv", uniquify=True, tag=f"{tag}_vv")
        nc.vector.tensor_scalar(out=vv[:], in0=mv[:, 1:2], scalar1=EPS,
                                scalar2=None, op0=AL.add)
        rstd = sb.tile([Q, 1], f32, name=f"{tag}_rstd", uniquify=True, tag=f"{tag}_rstd")
        ri = rstd[:].bitcast(i32)
        nc.vector.tensor_scalar(out=ri, in0=vv[:].bitcast(i32), scalar1=1,
                                scalar2=None, op0=AL.logical_shift_right)
        nc.vector.tensor_tensor(out=ri, in0=rsq_magic[:Q, :], in1=ri,
                                op=AL.subtract)
        tn = sb.tile([Q, 1], f32, name=f"{tag}_tn", uniquify=True, tag=f"{tag}_tn")
        nc.vector.scalar_tensor_tensor(out=tn[:], in0=rstd[:], scalar=rstd[:, 0:1],
                                       in1=vv[:], op0=AL.mult, op1=AL.mult)
        nc.vector.tensor_scalar(out=tn[:], in0=tn[:], scalar1=-0.5, scalar2=1.5,
                                op0=AL.mult, op1=AL.add)
        nc.vector.tensor_tensor(out=rstd[:], in0=rstd[:], in1=tn[:], op=AL.mult)
        return mv, rstd

    def ln(x_res_a, x_res_b, gname, bname, tag):
        """LayerNorm(a + b) -> new sbuf tile [Q, C]."""
        xs = ln_add(x_res_a, x_res_b, tag)
        mv, rstd = ln_stats(xs, tag)
        nc.vector.tensor_scalar(out=xs[:], in0=xs[:], scalar1=mv[:, 0:1],
                                scalar2=rstd[:, 0:1], op0=AL.subtract, op1=AL.mult)
        if not triv:
            nc.vector.tensor_tensor(out=xs[:], in0=xs[:], in1=SMALL[gname], op=AL.mult)
            nc.vector.tensor_tensor(out=xs[:], in0=xs[:], in1=SMALL[bname], op=AL.add)
        return xs

    # ---------------- phase 1: self attention + extra cross attention -------
    def kv_proj(kvT, nk, wv, wk, b_in_bf, tag):
        """K/V projections from transposed input: v_sb [nk, C], kT [DH, H*nk].

        wvk: [384, 768] chunks, cols 0:C = V-out, C:2C = K-out.  kT is laid
        out per head at base partition 0 (HW rejects matmuls whose operands
        sit at partition offset 64 when the psum out is a column slice).  The
        k bias is dropped: softmax over k is invariant to it.
        """
        v_ps = ps.tile([nk, C], f32, name=f"{tag}_vps", uniquify=True,
                       tag="acc", bufs=1)
        for i in range(3):
            nc.tensor.matmul(v_ps[:], lhsT=kvT[i], rhs=wv[i][:, 0:C],
                             start=(i == 0), stop=(triv and i == 2))
        if not triv:
            nc.tensor.matmul(v_ps[:], lhsT=ones_bf[:1, :nk],
                             rhs=b_in_bf[:1, 2 * C:3 * C], start=False, stop=True)
        v_sb = sb.tile([nk, C], bf16, name=f"{tag}_v", uniquify=True, tag=f"{tag}_v")
        nc.scalar.copy(out=v_sb[:], in_=v_ps[:])
        kT_ps = pse.tile([DH, H * nk], f32, name=f"{tag}_kTp", uniquify=True,
                         tag="kq", bufs=1)
        for h in range(H):
            for i in range(3):
                nc.tensor.matmul(kT_ps[:, h * nk:(h + 1) * nk],
                                 lhsT=wk[i][:, h * DH:(h + 1) * DH],
                                 rhs=kvT[i], start=(i == 0), stop=(i == 2))
        kT_sb = sb.tile([DH, H * nk], bf16, name=f"{tag}_kT", uniquify=True,
                        tag=f"{tag}_kT")
        nc.scalar.copy(out=kT_sb[:], in_=kT_ps[:])
        return v_sb, kT_sb

    def attention(xT, kv, nk, wq, b_in_bf, woT, b_out_bf, tag):
        """MHA in bf16 (f32 psum), scores kept transposed [k, (h q)].

        No max-subtraction in the softmax (scores are O(0.2) here); the
        per-(q,h) normalizer is computed by PE column sums and applied to the
        unnormalized o. Returns out-proj psum [Q, C] f32."""
        v_sb, kT_sb = kv
        qT_ps = pse.tile([DH, H * Q], f32, name=f"{tag}_qTp", uniquify=True,
                         tag="kq", bufs=1)
        for h in range(H):
            for i in range(3):
                nc.tensor.matmul(qT_ps[:, h * Q:(h + 1) * Q],
                                 lhsT=wq[i][:, h * DH:(h + 1) * DH],
                                 rhs=xT[i], start=(i == 0),
                                 stop=(triv and i == 2))
            if not triv:
                nc.tensor.matmul(qT_ps[:, h * Q:(h + 1) * Q],
                                 lhsT=b_in_bf[:1, h * DH:(h + 1) * DH],
                                 rhs=ones_bf[:1, :Q], start=False, stop=True)
        qT_sb = sb.tile([DH, H * Q], bf16, name=f"{tag}_qT", uniquify=True,
                        tag=f"{tag}_qT")
        nc.scalar.copy(out=qT_sb[:], in_=qT_ps[:])
        # transposed scores for all heads in one psum strip [nk, H*Q]
        sT_ps = pse.tile([nk, H * Q], f32, name=f"{tag}_sT", uniquify=True,
                         tag="kq", bufs=1)
        for h in range(H):
            nc.tensor.matmul(sT_ps[:, h * Q:(h + 1) * Q],
                             lhsT=kT_sb[:, h * nk:(h + 1) * nk],
                             rhs=qT_sb[:, h * Q:(h + 1) * Q],
                             start=True, stop=True)
        eT = sb.tile([nk, H * Q], bf16, name=f"{tag}_eT", uniquify=True,
                     tag=f"{tag}_eT")
        nc.scalar.activation(out=eT[:], in_=sT_ps[:], func=ACT.Exp,
                             bias=zcol[:nk, :1], scale=1.0 / np.sqrt(DH))
        if stop_after == "exp":
            return None
        # per-(q,h) sums via PE column reduction; unnormalized o
        ssum_ps = pse.tile([Q, H], f32, name=f"{tag}_ssp", uniquify=True, tag="mm")
        for h in range(H):
            nc.tensor.matmul(ssum_ps[:, h:h + 1], lhsT=eT[:, h * Q:(h + 1) * Q],
                             rhs=ones_bfc[:nk, :1], start=True, stop=True)
        rinv = sb.tile([Q, H], f32, name=f"{tag}_ri", uniquify=True, tag=f"{tag}_ri")
        nc.vector.reciprocal(out=rinv[:], in_=ssum_ps[:])
        o_ps = ps.tile([Q, C], f32, name=f"{tag}_ops", uniquify=True, tag="acc", bufs=1)
        for h in range(H):
            nc.tensor.matmul(o_ps[:, h * DH:(h + 1) * DH],
                             lhsT=eT[:, h * Q:(h + 1) * Q],
                             rhs=v_sb[:, h * DH:(h + 1) * DH],
                             start=True, stop=True)
        # fused psum->sbuf copy + per-(q,h) normalize
        o_sb = sb.tile([Q, C], bf16, name=f"{tag}_o", uniquify=True, tag=f"{tag}_o")
        nc.vector.tensor_tensor(
            out=o_sb[:].rearrange("q (h d) -> q h d", h=H),
            in0=o_ps[:].rearrange("q (h d) -> q h d", h=H),
            in1=rinv[:].rearrange("q (h one) -> q h one", one=1
                                  ).broadcast_to([Q, H, DH]), op=AL.mult)
        if stop_after == "onorm":
            return None
        oT = transpose_tiles(o_sb[:], f"{tag}_o", dtype=bf16, ceng=nc.scalar)
        t2_ps = ps.tile([Q, C], f32, name=f"{tag}_t2", uniquify=True, tag="acc", bufs=1)
        for i in range(3):
            nc.tensor.matmul(t2_ps[:], lhsT=oT[i][:], rhs=woT[i][:, :],
                             start=(i == 0), stop=(triv and i == 2))
        if not triv:
            nc.tensor.matmul(t2_ps[:], lhsT=ones_bf[:1, :Q], rhs=b_out_bf[:1, :],
                             start=False, stop=True)
        return t2_ps

    if stop_after == "pre":
        nc.sync.dma_start(out=io["out"][:], in_=tgt0[:])
        stack.close()
        return
    sa_kv = kv_proj(saT, Q, W["sa_v_wT"], W["sa_k_wT"], BF_B["sa_in_b"], "sa")
    if stop_after == "kv":
        nc.sync.dma_start(out=io["out"][:], in_=tgt0[:])
        stack.close()
        return
    t2 = attention(saT, sa_kv, Q, W["sa_q_wT"], BF_B["sa_in_b"],
                   W["sa_out_wT"], BF_B["sa_out_b"], "sa")
    # EA K/V prep emitted after the SA attention matmuls: wc_ea lands ~8us,
    # and putting it earlier would head-of-line-block SA's PE stream
    ea_kv = kv_proj(eaT, EXTRA,
                    [w[:, 0:C] for w in W["ea_vk_wT"]],
                    [w[:, C:2 * C] for w in W["ea_vk_wT"]],
                    BF_B["ea_in_b"], "ea")
    if stop_after in ("exp", "onorm", "t2"):
        nc.sync.dma_start(out=io["out"][:], in_=tgt0[:])
        stack.close()
        return
    tgt1 = ln(tgt0[:], t2[:], "ln2_g", "ln2_b", "ln2")
    if stop_after == "sa":
        nc.sync.dma_start(out=io["out"][:], in_=tgt1[:])
        stack.close()
        return
    t1T = transpose_tiles(tgt1[:], "ea_x", dtype=bf16)
    t2 = attention([t[:] for t in t1T], ea_kv, EXTRA, W["ea_q_wT"],
                   BF_B["ea_in_b"], W["ea_out_wT"], BF_B["ea_out_b"], "ea")
    if triv:
        # LNE folded into the offset/aw projection: the transposes and
        # matmuls run on the raw residual x = tgt1 + t2 while the moment
        # statistics compute in parallel; the psum read-out applies
        # oa = rstd*(x@W) - (mu*rstd)*colsum(W) + b, which equals
        # (LN(x) @ W + b) exactly for unit gain / zero LN bias.
        xs2 = ln_add(tgt1[:], t2[:], "lne")
        src_t2 = xs2
    else:
        tgt2 = ln(tgt1[:], t2[:], "lne_g", "lne_b", "lne")
        src_t2 = tgt2
    if stop_after == "ea":
        nc.sync.dma_start(out=io["out"][:], in_=tgt1[:])
        stack.close()
        return

    # ---------------- phase 2: sampling locations ----------------
    # Everything below runs directly in the (p = v*64 + q) partition layout:
    # the y-slot (v) dependent values are baked into host tables (qpk2), so no
    # cross-partition shuffles are needed. x is row-duplicated into both
    # partition halves via a broadcast copy of its transposes.
    tgt2dT = []
    for i in range(3):
        pt = pse.tile([128, Q], f32, name=f"t2T_{i}", uniquify=True, tag="mm", bufs=2)
        nc.tensor.transpose(out=pt[:], in_=src_t2[:, i * 128:(i + 1) * 128],
                            identity=ident[:Q, :Q])
        st = sb.tile([128, 128], bf16, name=f"t2dT_{i}", uniquify=True, tag=f"t2dT{i}")
        nc.vector.tensor_copy(
            out=st[:].rearrange("p (u q) -> p u q", u=2),
            in_=pt[:, 0:Q].rearrange("p (one q) -> p one q", one=1
                                     ).broadcast_to([128, 2, Q]))
        tgt2dT.append(st)
    oa_ps = ps.tile([128, 3 * NCOL], f32, name="oa_ps", tag="acc", bufs=1)
    for i in range(3):
        nc.tensor.matmul(oa_ps[:], lhsT=tgt2dT[i][:], rhs=W["offaw_wT"][i][:, :],
                         start=(i == 0), stop=(triv and i == 2))
    oa = sb.tile([128, 3 * NCOL], f32, name="oa")
    if triv:
        mv2, rstd2 = ln_stats(xs2, "lne")
        mustd = sb.tile([Q, 2], f32, name="mustd")
        nc.vector.tensor_tensor(out=mustd[:, 0:1], in0=mv2[:, 0:1],
                                in1=rstd2[:], op=AL.mult)
        nc.vector.tensor_copy(out=mustd[:, 1:2], in_=rstd2[:])
        dup_ps = pse.tile([128, 2], f32, name="dup_ps", tag="mm")
        nc.tensor.matmul(dup_ps[:], lhsT=maskT, rhs=mustd[:], start=True,
                         stop=True)
        ms128 = sb.tile([128, 2], f32, name="ms128")
        nc.vector.tensor_copy(out=ms128[:], in_=dup_ps[:])
        nc.vector.tensor_scalar(out=oa[:], in0=oa_ps[:],
                                scalar1=ms128[:, 1:2], scalar2=None, op0=AL.mult)
        tmp180 = sb.tile([128, 3 * NCOL], f32, name="tmp180")
        nc.vector.scalar_tensor_tensor(out=tmp180[:], in0=SMALL["wsum2"],
                                       scalar=ms128[:, 0:1], in1=SMALL["ob2"],
                                       op0=AL.mult, op1=AL.subtract)
        nc.vector.tensor_tensor(out=oa[:], in0=oa[:], in1=tmp180[:],
                                op=AL.subtract)
        # materialize tgt2 = LN(x) for the later residual (off critical path)
        nc.vector.tensor_scalar(out=xs2[:], in0=xs2[:], scalar1=mv2[:, 0:1],
                                scalar2=rstd2[:, 0:1], op0=AL.subtract,
                                op1=AL.mult)
        tgt2 = xs2
    else:
        nc.tensor.matmul(oa_ps[:], lhsT=ones_bf[:1, :128],
                         rhs=BF_B["offaw_b"][:1, :], start=False, stop=True)
        nc.vector.tensor_copy(out=oa[:], in_=oa_ps[:])
    off4 = oa[:, :2 * NCOL].rearrange("p (c two) -> p c two", two=2)
    def t128(tag):
        return sb.tile([128, NCOL], f32, name=tag, uniquify=True, tag=tag)

    def floor_pair(val, tag, eng=None):
        """val is pre-shifted by (-0.5 + eps) via the host xb2/yb2 bases:
        rne(val) == floor of the true coordinate (exact except within eps of
        an integer, where bilinear continuity bounds the output error), and
        fr == frac - 0.5 + eps; consumers fold the constant back."""
        eng = eng or nc.vector
        fl = t128(f"{tag}_fl")
        eng.tensor_scalar(out=fl[:], in0=val, scalar1=MAGIC, scalar2=MAGIC,
                          op0=AL.add, op1=AL.subtract)
        fr = t128(f"{tag}_fr")
        eng.tensor_tensor(out=fr[:], in0=val, in1=fl[:], op=AL.subtract)
        return fl, fr

    # x side (identical in both partition halves): slot weights for u=0/1
    xx = t128("xx")
    nc.vector.tensor_tensor(out=xx[:], in0=SMALL["xb2"], in1=off4[:, :, 0],
                            op=AL.add)
    x0, fx = floor_pair(xx[:], "fx")
    xs = t128("xs")
    nc.vector.tensor_scalar(out=xs[:], in0=x0[:], scalar1=0.0, scalar2=None,
                            op0=AL.max)
    nc.vector.tensor_tensor(out=xs[:], in0=xs[:], in1=SMALL["wm22"], op=AL.min)
    # y side: v lives in the partition dim; single slot weight per partition
    yy = t128("yy")
    nc.vector.tensor_tensor(out=yy[:], in0=SMALL["yb2"], in1=off4[:, :, 1],
                            op=AL.add)
    y0, fy = floor_pair(yy[:], "fy")
    ys = t128("ys")
    nc.vector.tensor_scalar(out=ys[:], in0=y0[:], scalar1=0.0, scalar2=None,
                            op0=AL.max)
    nc.vector.tensor_tensor(out=ys[:], in0=ys[:], in1=SMALL["hm22"], op=AL.min)
    # row indices straight into the wrap source [128, 84]:
    # cols 0:60 = A rows (levels 0-3 used; l4 cols present but unused),
    # cols 60:72 = l4 B-window rows, 72:84 = l4 C-window rows
    idxsrc = sb.tile([128, NIX], f32, name="idxsrc")
    ifl = idxsrc[:, 0:NCOL]
    nc.vector.tensor_tensor(out=ifl, in0=ys[:], in1=SMALL["wtab2"], op=AL.mult)
    nc.vector.tensor_tensor(out=ifl, in0=ifl, in1=xs[:], op=AL.add)
    nc.vector.tensor_tensor(out=ifl, in0=ifl, in1=SMALL["base2"], op=AL.add)
    # A and B/C wrapped indices live in SEPARATE tiles: the A gathers'
    # descriptor generation must not wait (whole-tile dependency) on the
    # level-4 wrap copy, which can queue behind coefficient builds on DVE
    idx16a = [sb.tile([128, JH * 8], mybir.dt.int16, name=f"idx16a{h}",
                      uniquify=True, tag="i16a", bufs=3) for h in range(HG)]
    idx16bc = sb.tile([128, (NIX - NCOL) * 8], mybir.dt.int16, name="idx16bc")

    def wrap_cols(dst, c0, c1, tag):
        """Wrap idxsrc[:, c0:c1] into dst (int16, [128, (c1-c0)*8])."""
        n = c1 - c0
        dv = dst[:].rearrange("p (j g) -> p j g", g=8)
        for half in range(2):
            wr_ps = pse.tile([128, 4 * NIX], f32, name=f"wrp{tag}{half}",
                             uniquify=True, tag="mm")
            for gg in range(4):
                g = half * 4 + gg
                nc.tensor.matmul(wr_ps[:, gg * n:(gg + 1) * n],
                                 lhsT=rmat[:, g * 128:(g + 1) * 128],
                                 rhs=idxsrc[:, c0:c1], start=True, stop=True)
            nc.scalar.copy(
                out=dv[:, :, half * 4:(half + 1) * 4],
                in_=wr_ps[:, :4 * n].rearrange("p (g j) -> p j g", j=n))

    # per-head index tiles: head h's A gathers unblock as soon as its own
    # 20 columns are wrapped (whole-tile dependencies otherwise serialize
    # every gather behind the full wrap)
    for h in range(HG):
        wrap_cols(idx16a[h], h * JH, (h + 1) * JH, f"a{h}")

    # level-4 window split: B rows [16660, 49364), C rows [49364, 66836).
    # Every l4 (point, v) is gathered in both windows with complementary
    # gate coefficients; clipped rel indices keep reads in range.
    i4 = idxsrc[:, 0:NCOL].rearrange("p (h j) -> p h j", h=HG)[:, :, 16:20]
    iB = idxsrc[:, NCOL:NCOL + HG * 4].rearrange("p (h j) -> p h j", h=HG)
    nc.vector.tensor_scalar(out=iB, in0=i4, scalar1=-float(WROW_B),
                            scalar2=0.0, op0=AL.add, op1=AL.max)
    nc.vector.tensor_scalar(out=iB, in0=iB, scalar1=float(NW_B - 1), scalar2=None,
                            op0=AL.min)
    iC = idxsrc[:, NCOL + HG * 4:NIX].rearrange("p (h j) -> p h j", h=HG)
    nc.vector.tensor_scalar(out=iC, in0=i4, scalar1=-float(WROW_C),
                            scalar2=0.0, op0=AL.add, op1=AL.max)
    nc.vector.tensor_scalar(out=iC, in0=iC, scalar1=float(NW_C - 1), scalar2=None,
                            op0=AL.min)
    wrap_cols(idx16bc, NCOL, NIX, "bc")

    # Within a gather whose first idxsrc col is J0, index i = (j-J0)*128 + p
    # is read from idx16[i%16 (any replica row), j*8 + (i%128)//16].  The 8
    # selection matmuls R_g (R_g[p_in, p_out] = [p_in == g*16 + p_out%16])
    # produce out_g[p, j] = idxsrc[g*16 + p%16, j], i.e. the wrapped+replicated
    # layout directly in PSUM; two strided int16 copies interleave g into the
    # final column order.

    # PE warm-up: ~2.5us of dependency-pinned junk transposes right after the
    # wrap keeps the tensor engine's p-state ramped into the combine burst
    # (cold restarts cost 3.7x per matmul until 3us of continuous execution)
    for j in range(28):
        jp = pse.tile([128, 128], f32, name=f"warm{j}", uniquify=True,
                      tag="mm", bufs=2)
        nc.tensor.transpose(out=jp[:NIX, :128], in_=idxsrc[:, 0:NIX],
                            identity=ident[:, :])

    # unnormalized attention weights: exp only; 1/sum applied on the psum
    # read-out of each head's combined sum
    eaw = sb.tile([128, NCOL], f32, name="eaw")
    nc.scalar.activation(out=eaw[:], in_=oa[:, 2 * NCOL:3 * NCOL], func=ACT.Exp,
                         bias=zcol[:128, :1], scale=1.0)
    awsum = sb.tile([128, HG], f32, name="awsum")
    nc.vector.reduce_sum(
        out=awsum[:].rearrange("p (h one) -> p h one", one=1),
        in_=eaw[:].rearrange("p (h j) -> p h j", h=HG), axis=AX.X)
    rinv_aw = sb.tile([128, HG], f32, name="rinv_aw")
    nc.vector.reciprocal(out=rinv_aw[:], in_=awsum[:])

    # corner-weight chains: comparisons must stay on DVE (the Pool ISA
    # rejects is_ge/is_le/is_equal); the mult/add arithmetic runs on the
    # otherwise idle Pool engine in parallel with DVE
    gex = t128("gex")
    nc.vector.tensor_scalar(out=gex[:], in0=x0[:], scalar1=0.0, scalar2=None,
                            op0=AL.is_ge)
    lex = t128("lex")
    nc.vector.tensor_tensor(out=lex[:], in0=x0[:], in1=SMALL["wm22"], op=AL.is_le)
    em1x = t128("em1x")
    nc.vector.tensor_scalar(out=em1x[:], in0=x0[:], scalar1=-1.0, scalar2=None,
                            op0=AL.is_equal)
    eWx = t128("eWx")
    nc.vector.tensor_tensor(out=eWx[:], in0=x0[:], in1=SMALL["wm12"], op=AL.is_equal)
    gey = t128("gey")
    nc.vector.tensor_scalar(out=gey[:], in0=y0[:], scalar1=0.0, scalar2=None,
                            op0=AL.is_ge)
    ley = t128("ley")
    nc.vector.tensor_tensor(out=ley[:], in0=y0[:], in1=SMALL["hm22"], op=AL.is_le)
    edgy = t128("edgy")
    nc.vector.tensor_tensor(out=edgy[:], in0=y0[:], in1=SMALL["etab2"],
                            op=AL.is_equal)
    veng = nc.gpsimd
    inbx = t128("inbx")
    veng.tensor_tensor(out=inbx[:], in0=gex[:], in1=lex[:], op=AL.mult)
    omfx = t128("omfx")
    veng.tensor_scalar(out=omfx[:], in0=fx[:], scalar1=-1.0,
                       scalar2=0.5 + 2.0 ** -9, op0=AL.mult, op1=AL.add)
    wx0 = t128("wx0")
    veng.tensor_tensor(out=wx0[:], in0=omfx[:], in1=inbx[:], op=AL.mult)
    tmpx = t128("tmpx")
    nc.vector.scalar_tensor_tensor(out=tmpx[:], in0=fx[:],
                                   scalar=0.5 - 2.0 ** -9, in1=em1x[:],
                                   op0=AL.add, op1=AL.mult)
    veng.tensor_tensor(out=wx0[:], in0=wx0[:], in1=tmpx[:], op=AL.add)
    wx1 = t128("wx1")
    nc.vector.scalar_tensor_tensor(out=wx1[:], in0=fx[:],
                                   scalar=0.5 - 2.0 ** -9, in1=inbx[:],
                                   op0=AL.add, op1=AL.mult)
    veng.tensor_tensor(out=tmpx[:], in0=omfx[:], in1=eWx[:], op=AL.mult)
    veng.tensor_tensor(out=wx1[:], in0=wx1[:], in1=tmpx[:], op=AL.add)
    inby = t128("inby")
    veng.tensor_tensor(out=inby[:], in0=gey[:], in1=ley[:], op=AL.mult)
    # alpha = fy*sgn + vcol  (v=0 -> 1-fy, v=1 -> fy); wy = a*inb + (1-a)*edge
    alp = t128("alp")
    veng.tensor_scalar(out=alp[:], in0=fy[:], scalar1=SMALL["sgn2"][:, 0:1],
                            scalar2=SMALL["vcol2"][:, 0:1], op0=AL.mult, op1=AL.add)
    bet = t128("bet")
    veng.tensor_scalar(out=bet[:], in0=alp[:], scalar1=-1.0, scalar2=1.0,
                            op0=AL.mult, op1=AL.add)
    wy = t128("wy")
    veng.tensor_tensor(out=wy[:], in0=alp[:], in1=inby[:], op=AL.mult)
    veng.tensor_tensor(out=bet[:], in0=bet[:], in1=edgy[:], op=AL.mult)
    veng.tensor_tensor(out=wy[:], in0=wy[:], in1=bet[:], op=AL.add)

    # coefficients ctile[p, (h j u)] = wy * wx_u * eaw (aw-unnormalized)
    ctile = sb.tile([128, 2 * NCOL], f32, name="ctile")
    ct3 = ctile[:].rearrange("p (j u) -> p j u", u=2)
    cdup = t128("cdup")
    nc.vector.tensor_tensor(out=cdup[:], in0=wy[:], in1=eaw[:], op=AL.mult)
    nc.vector.tensor_tensor(out=ct3[:, :, 0], in0=cdup[:], in1=wx0[:], op=AL.mult)
    nc.vector.tensor_tensor(out=ct3[:, :, 1], in0=cdup[:], in1=wx1[:], op=AL.mult)

    gB4 = sb.tile([128, HG * 4], f32, name="gB4")
    nc.vector.tensor_scalar(out=gB4[:].rearrange("p (h j) -> p h j", h=HG),
                            in0=i4, scalar1=float(WROW_C), scalar2=None,
                            op0=AL.is_lt)
    # gated l4 coefficients
    c_l4 = ctile[:].rearrange("p (h j u) -> p h j u", h=HG, u=2)[:, :, 16:20, :]
    gB4b = gB4[:].rearrange("p (h j one) -> p h j one", h=HG, one=1
                            ).broadcast_to([128, HG, 4, 2])
    ctileB = sb.tile([128, HG * 8], f32, name="ctileB")
    ctileC = sb.tile([128, HG * 8], f32, name="ctileC")
    cB4 = ctileB[:].rearrange("p (h j u) -> p h j u", h=HG, u=2)
    cC4 = ctileC[:].rearrange("p (h j u) -> p h j u", h=HG, u=2)
    nc.vector.tensor_tensor(out=cB4, in0=c_l4, in1=gB4b, op=AL.mult)
    nc.vector.tensor_tensor(out=cC4, in0=c_l4, in1=cB4, op=AL.subtract)

    if stop_after == "phase2":
        dbg = dram.tile([128, JH * 8], mybir.dt.int16, name="dbg16")
        nc.sync.dma_start(out=dbg[:], in_=idx16a[0][:])
        nc.sync.dma_start(out=io["out"][:], in_=tgt2[:])
        stack.close()
        return

    # ---------------- phase 3: gather + combine + per-head projection ----------
    # 5 merged gathers (small B/C windows first so per-head combines can
    # finish — and per-head projections start — as soon as A_h lands):
    #   B_all/C_all: all heads' level-4 window tiles, 1536 idx each
    #   A_h:         per-head levels 0-3, 2048 idx
    # every gather stays <= 1024 descriptors (the SWDGE ring hangs on HW
    # beyond that); A tiles are filled by two 1024-descriptor gathers into
    # halves of one buffer
    def gather(out_view, base, nwin, c0, ncols):
        if c0 < NCOL:
            it, cb = idx16a[c0 // JH], c0 % JH
        else:
            it, cb = idx16bc, c0 - NCOL
        nc.gpsimd.dma_gather(
            out_ap=out_view,
            in_ap=bass.AP(io["srcflat_f8"].tensor, base * CP,
                          [[CP, nwin], [1, 2 * CP]]),
            idxs_ap=it[:, cb * 8:(cb + ncols) * 8],
            num_idxs=ncols * 128, num_idxs_reg=ncols * 128,
            elem_size=2 * CP, elem_step=CP)

    # per-head gather order [A_h x2, B_h, C_h]: head h's combine and fused
    # projection can run while head h+1 still streams
    gA, gBh, gCh = [], [], []
    for h in range(HG):
        ga = sb.tile([128, 16 * 2 * CP], f8, name=f"gA{h}", uniquify=True,
                     tag="gA", bufs=3)
        gav = ga[:].rearrange("p (b e) -> p b e", e=2 * CP)
        for k in range(2):
            gather(gav[:, k * 8:(k + 1) * 8, :], 0, WROW_B, h * JH + k * 8, 8)
        gA.append(ga)
        gb = sb.tile([128, 4 * 2 * CP], f8, name=f"gB{h}", uniquify=True,
                     tag="gBC", bufs=6)
        gather(gb[:].rearrange("p (b e) -> p b e", e=2 * CP),
               WROW_B, NW_B, NCOL + h * 4, 4)
        gBh.append(gb)
        gc = sb.tile([128, 4 * 2 * CP], f8, name=f"gC{h}", uniquify=True,
                     tag="gBC", bufs=6)
        gather(gc[:].rearrange("p (b e) -> p b e", e=2 * CP),
               WROW_C, NW_C, NCOL + HG * 4 + h * 4, 4)
        gCh.append(gc)

    # wide coefficient tiles CT[p, (k q)] = mask[p, q] * coef[p, k] built in
    # one DVE op per gather tile (bf16 for 2x DVE throughput)
    def ct_wide(coef_sl, ncols, tag):
        t = sb.tile([128, ncols * Q], bf16, name=tag, uniquify=True, tag=tag)
        nc.vector.tensor_tensor(
            out=t[:].rearrange("p (k q) -> p k q", q=Q),
            in0=mask_bf[:].rearrange("p (one q) -> p one q", one=1
                                     ).broadcast_to([128, ncols, Q]),
            in1=coef_sl.rearrange("p (k one) -> p k one", one=1
                                  ).broadcast_to([128, ncols, Q]),
            op=AL.mult)
        return t

    # all coefficient tiles are emitted before any combine/tail work so the
    # in-order DVE queue never stalls a later head's coefficients behind an
    # earlier head's gather-dependent tail ops
    ctA = [ct_wide(ctile[:, h * 2 * JH:h * 2 * JH + 32], 32, f"ctA{h}")
           for h in range(HG)]
    ctB = [ct_wide(ctileB[:, h * 8:(h + 1) * 8], 8, f"ctB{h}")
           for h in range(HG)]
    ctC = [ct_wide(ctileC[:, h * 8:(h + 1) * 8], 8, f"ctC{h}")
           for h in range(HG)]

    oh_ps = [ps.tile([Q, C], f32, name=f"oh_ps{h}", uniquify=True, tag="oh",
                     bufs=3) for h in range(HG)]
    nmm = [0] * HG
    NMM = 2 * 16 + 2 * 4 + 2 * 4

    def combine(h, gt, nb, bbase, ct):
        """oh_ps[h] += sum over nb (b,u) units from gather tile gt."""
        g3 = gt[:].rearrange("p (b e) -> p b e", e=2 * CP)
        for b in range(nb):
            for u in range(2):
                k = 2 * b + u
                nc.tensor.matmul(oh_ps[h][:], lhsT=ct[:, k * Q:(k + 1) * Q],
                                 rhs=g3[:, bbase + b, u * CP:u * CP + C],
                                 start=(nmm[h] == 0), stop=(nmm[h] == NMM - 1))
                nmm[h] += 1

    if triv:
        # per-head: normalize on psum read-out, transpose, and accumulate the
        # fused (val @ out) projection into one [Q, C] partial
        mo_ps = ps.tile([Q, C], f32, name="mo_ps", tag="acc", bufs=1)
        first_mo = [True]

        def head_tail(h):
            oh_sb = sb.tile([Q, C], bf16, name=f"oh_sb{h}", uniquify=True,
                            tag="oh_sb")
            nc.vector.tensor_scalar(out=oh_sb[:], in0=oh_ps[h][:],
                                    scalar1=rinv_aw[0:Q, h:h + 1], scalar2=None,
                                    op0=AL.mult)
            ohT = transpose_tiles(oh_sb[:], f"ohT{h}", dtype=bf16,
                                  ceng=nc.scalar)
            for i in range(3):
                nc.tensor.matmul(mo_ps[:], lhsT=ohT[i][:],
                                 rhs=W["wf_wT"][i][:, h * C:(h + 1) * C],
                                 start=(first_mo[0] and i == 0),
                                 stop=(h == HG - 1 and i == 2))
            first_mo[0] = False

        # head h's tail is emitted after head h+1's combine: the in-order PE
        # queue then overlaps the oh_sb/transpose roundtrip with real matmuls
        for h in range(HG):
            combine(h, gA[h], 16, 0, ctA[h])
            combine(h, gBh[h], 4, 0, ctB[h])
            combine(h, gCh[h], 4, 0, ctC[h])
            if h >= 1:
                head_tail(h - 1)
        head_tail(HG - 1)
        mo_sb = sb.tile([Q, C], f32, name="mo_sb")
        nc.vector.tensor_copy(out=mo_sb[:], in_=mo_ps[:])
        # ------- phase 4: 2-rank AllReduce of the partial output -------
        mo_full = sb.tile([Q, C], f32, name="mo_full")
        if use_ag:
            cc_in = dram.tile([Q, C], f32, name="cc_in")
            cc_out = dram.tile([Q, C], f32, name="cc_out")
            nc.gpsimd.dma_start(out=cc_in[:], in_=mo_sb[:])
            nc.gpsimd.collective_compute(
                "AllReduce", mybir.AluOpType.add,
                replica_groups=[[0, 1], [2, 3], [4, 5], [6, 7]],
                ins=[cc_in[:].opt()], outs=[cc_out[:].opt()])
            nc.sync.dma_start(out=mo_full[:], in_=cc_out[:])
        else:
            mo_full = mo_sb
    else:
        heads_sb = sb.tile([Q, HG * DH], f32, name="heads_sb")
        for h in range(HG):
            combine(h, gA[h], 16, 0, ctA[h])
            combine(h, gBh[h], 4, 0, ctB[h])
            combine(h, gCh[h], 4, 0, ctC[h])
            # sum of coefficients (for value-bias correction): swT [1, Q]
            red = sb.tile([128, 1], f32, name=f"red{h}", uniquify=True, tag="red")
            nc.vector.reduce_sum(out=red[:],
                                 in_=ctile[:, h * 2 * JH:(h + 1) * 2 * JH],
                                 axis=AX.X)
            nc.vector.tensor_tensor(out=red[:], in0=red[:],
                                    in1=rinv_aw[:, h:h + 1], op=AL.mult)
            swT_ps = pse.tile([1, Q], f32, name=f"swTp{h}", uniquify=True, tag="mm")
            nc.tensor.matmul(swT_ps[:], lhsT=red[:], rhs=mask_f, start=True,
                             stop=True)
            swT = sb.tile([1, Q], bf16, name=f"swT{h}", uniquify=True, tag="swT")
            nc.vector.tensor_copy(out=swT[:], in_=swT_ps[:])
            oh_sb = sb.tile([Q, C], bf16, name=f"oh_sb{h}", uniquify=True,
                            tag="oh_sb")
            nc.vector.tensor_scalar(out=oh_sb[:], in0=oh_ps[h][:],
                                    scalar1=rinv_aw[0:Q, h:h + 1], scalar2=None,
                                    op0=AL.mult)
            ohT = transpose_tiles(oh_sb[:], f"ohT{h}", dtype=bf16)
            pj_ps = pse.tile([Q, DH], f32, name=f"pj{h}", uniquify=True, tag="mm")
            for i in range(3):
                nc.tensor.matmul(pj_ps[:], lhsT=ohT[i][:],
                                 rhs=W["val_wT_g"][i][:, h * DH:(h + 1) * DH],
                                 start=(i == 0), stop=False)
            nc.tensor.matmul(pj_ps[:], lhsT=swT[:1, :],
                             rhs=BF_B["val_b_g"][:1, h * DH:(h + 1) * DH],
                             start=False, stop=True)
            nc.vector.tensor_copy(out=heads_sb[:, h * DH:(h + 1) * DH],
                                  in_=pj_ps[:])
        # ------- phase 4: exchange head groups (2-rank AllGather) -------
        headsfull = sb.tile([Q, C], f32, name="headsfull")
        if use_ag:
            cc_in = dram.tile([Q, HG * DH], f32, name="cc_in")
            cc_out = dram.tile([2 * Q, HG * DH], f32, name="cc_out")
            nc.gpsimd.dma_start(out=cc_in[:], in_=heads_sb[:])
            nc.gpsimd.collective_compute(
                "AllGather", mybir.AluOpType.bypass,
                replica_groups=[[0, 1], [2, 3], [4, 5], [6, 7]],
                ins=[cc_in[:].opt()], outs=[cc_out[:].opt()])
            nc.sync.dma_start(out=headsfull[:, 0:HG * DH], in_=cc_out[0:Q, :])
            nc.sync.dma_start(out=headsfull[:, HG * DH:C], in_=cc_out[Q:2 * Q, :])
        else:
            nc.vector.tensor_copy(out=headsfull[:, 0:HG * DH], in_=heads_sb[:])
            nc.vector.tensor_copy(out=headsfull[:, HG * DH:C], in_=heads_sb[:])
        hfT = transpose_tiles(headsfull[:], "hfT", dtype=bf16)
        mo_ps = ps.tile([Q, C], f32, name="mo_ps", tag="acc", bufs=1)
        for i in range(3):
            nc.tensor.matmul(mo_ps[:], lhsT=hfT[i][:], rhs=W["out_wT"][i][:, :],
                             start=(i == 0), stop=False)
        nc.tensor.matmul(mo_ps[:], lhsT=ones_bf[:1, :Q], rhs=BF_B["out_b"][:1, :],
                         start=False, stop=True)
        mo_full = mo_ps

    # ---------------- phase 5: LN + FFN + LN ----------------
    tgt3 = ln(tgt2[:], mo_full[:], "ln1_g", "ln1_b", "ln1")

    tgt3T = transpose_tiles(tgt3[:], "t3T", dtype=bf16)
    # h1 computed directly in transposed layout (chunks of 128 DFF rows) so
    # no [Q, DFF] transpose stage is needed before the second matmul
    h1T = []
    for m in range(8):
        f1_ps = pse.tile([128, Q], f32, name=f"f1_{m}", uniquify=True, tag="mm")
        for i in range(3):
            nc.tensor.matmul(f1_ps[:], lhsT=W["ffn_w1T"][i][:, m * 128:(m + 1) * 128],
                             rhs=tgt3T[i][:], start=(i == 0),
                             stop=(triv and i == 2))
        if not triv:
            nc.tensor.matmul(f1_ps[:], lhsT=BF_B["ffn_b1"][:1, m * 128:(m + 1) * 128],
                             rhs=ones_bf[:1, :Q], start=False, stop=True)
        t = sb.tile([128, Q], bf16, name=f"h1T_{m}", uniquify=True, tag="h1T",
                    bufs=3)
        if m % 2 == 0:
            nc.scalar.activation(out=t[:], in_=f1_ps[:], func=ACT.Relu,
                                 bias=zcol[:128, :1])
        else:
            nc.vector.tensor_scalar(out=t[:], in0=f1_ps[:], scalar1=0.0,
                                    scalar2=None, op0=AL.max)
        h1T.append(t)
    f2_ps = ps.tile([Q, C], f32, name="f2_ps", tag="acc", bufs=1)
    for i in range(8):
        nc.tensor.matmul(f2_ps[:], lhsT=h1T[i][:], rhs=W["ffn_w2T"][i][:, :],
                         start=(i == 0), stop=(triv and i == 7))
    if not triv:
        nc.tensor.matmul(f2_ps[:], lhsT=ones_bf[:1, :Q], rhs=BF_B["ffn_b2"][:1, :],
                         start=False, stop=True)
    out_sb = ln(tgt3[:], f2_ps[:], "ln3_g", "ln3_b", "ln3")
    nc.sync.dma_start(out=io["out"][:], in_=out_sb[:])
    stack.close()


def _build(n_devices=N_CORES, use_ag=True, triv=True, loop=1, stop_after=None):
    import concourse.bacc as bacc
    import concourse.mybir as mybir
    import concourse.tile as tile
    from concourse._compat import axon_active
    f32 = mybir.dt.float32
    nc = bacc.Bacc("TRN2", target_bir_lowering=False, debug=not axon_active(),
                   num_devices=n_devices)
    io = {}
    for name, shape in [("tgt_in", [Q, C]),
                        ("mpk", [128, Q + 9 * 128]),
                        ("qpk2", [128, NQPK2])]:
        io[name] = nc.dram_tensor(name, shape, f32, kind="ExternalInput").ap()
    if not triv:
        io["qpk"] = nc.dram_tensor("qpk", [Q, NQPK], f32, kind="ExternalInput").ap()
    io["srcflat_f8"] = nc.dram_tensor("srcflat_f8", [S, CP], mybir.dt.float8e4,
                                      kind="ExternalInput").ap()
    io["biasrow"] = nc.dram_tensor("biasrow", [1, NBIAS], mybir.dt.bfloat16,
                                   kind="ExternalInput").ap()
    for cname, _, items in wchunks(triv):
        io[cname] = nc.dram_tensor(cname, [128, _chunk_cols(items)],
                                   mybir.dt.bfloat16, kind="ExternalInput").ap()
    io["out"] = nc.dram_tensor("out", [Q, C], f32, kind="ExternalOutput").ap()

    with tile.TileContext(nc) as tc:
        for _ in range(loop):
            _emit(tc, io, use_ag=use_ag, triv=triv, stop_after=stop_after)
    nc.compile()
    return nc


def _is_triv(inp):
    zeros = ["sa_in_b", "sa_out_b", "ea_in_b", "ea_out_b", "ms_val_b",
             "ms_out_b", "ffn_b1", "ffn_b2", "ln2_b", "lne_b", "ln1_b", "ln3_b"]
    ones = ["ln2_g", "lne_g", "ln1_g", "ln3_g"]
    return (all(not np.any(np.asarray(inp[n])) for n in zeros)
            and all(np.all(np.asarray(inp[n]) == 1.0) for n in ones))


def make_in_maps(inputs, triv):
    """Build the 8 per-core input maps from the full problem inputs (numpy)."""
    import ml_dtypes
    inp = {k: np.ascontiguousarray(np.asarray(v, dtype=np.float32))
           if not k.startswith("src_") or k == "src" else np.asarray(v)
           for k, v in inputs.items()}
    lsi = np.asarray(inputs["src_level_start_index"]).astype(np.int64)
    spat = np.asarray(inputs["src_spatial_shapes"]).astype(np.int64)
    Wl = spat[:, 1].astype(np.float32)
    Hl = spat[:, 0].astype(np.float32)
    lcol = np.tile(np.repeat(np.arange(L), PTS), HG)  # [NCOL]
    mask = np.zeros((128, Q), np.float32)
    mask[np.arange(128), np.arange(128) % Q] = 1.0

    def wT(a):
        return np.ascontiguousarray(a.T.astype(np.float32))

    def repl(a):
        return np.ascontiguousarray(
            np.broadcast_to(a.reshape(1, -1), (Q, C)).astype(np.float32))

    # R_g[p_in, p_out] = 1 iff p_in == g*16 + p_out%16 (PE wrap matrices)
    pin = np.arange(128)[:, None]
    pout = np.arange(128)[None, :]
    mpk = np.zeros((128, Q + 9 * 128), np.float32)
    mpk[:, :Q] = mask
    for g in range(8):
        mpk[:, Q + g * 128:Q + (g + 1) * 128] = (
            pin == g * 16 + pout % 16).astype(np.float32)
    mpk[:Q, Q + 8 * 128:] = mask.T  # [Q, 128] partition-duplication matrix

    def pack_chunk(items, warrs):
        """warrs: name -> wT array [K, N]; returns [128, cols] bf16."""
        blocks = []
        for name, k, n in items:
            a = warrs[name]
            assert a.shape == (k, n), (name, a.shape, (k, n))
            for i in range(k // 128):
                blocks.append(a[i * 128:(i + 1) * 128])
        return np.ascontiguousarray(
            np.concatenate(blocks, axis=1).astype(ml_dtypes.bfloat16))

    # fp8 src with rows padded to CP bytes (exact 2-row gather descriptors)
    srcpads = []
    for b in range(B):
        sp = np.zeros((S, CP), ml_dtypes.float8_e4m3)
        sp[:, :C] = np.asarray(inp["src"][b]).astype(ml_dtypes.float8_e4m3)
        srcpads.append(sp)

    sa_wT = wT(inp["sa_in_w"])
    ea_wT = wT(inp["ea_in_w"])
    WCH = wchunks(triv)

    in_maps = []
    for c in range(N_CORES):
        b, g = c // 2, c % 2
        heads = range(HG * g, HG * g + HG)
        vr = np.asarray(inp["src_valid_ratios"])[b]  # [L, 2]
        off_rows = np.concatenate([np.arange(h * L * PTS * 2, (h + 1) * L * PTS * 2)
                                   for h in heads])
        aw_rows = np.concatenate([np.arange(h * L * PTS, (h + 1) * L * PTS)
                                  for h in heads])
        offaw_w = np.concatenate([inp["ms_off_w"][off_rows],
                                  inp["ms_attn_w"][aw_rows]], axis=0)  # [180, C]
        offaw_b = np.concatenate([inp["ms_off_b"][off_rows],
                                  inp["ms_attn_b"][aw_rows]])
        vcols = np.concatenate([np.arange(h * DH, (h + 1) * DH) for h in heads])
        warrs = dict(
            sa_v_wT=np.ascontiguousarray(sa_wT[:, 2 * C:3 * C]),
            sa_k_wT=np.ascontiguousarray(sa_wT[:, C:2 * C]),
            sa_q_wT=np.ascontiguousarray(sa_wT[:, 0:C]),
            sa_out_wT=wT(inp["sa_out_w"]),
            ea_vk_wT=np.concatenate([ea_wT[:, 2 * C:3 * C], ea_wT[:, C:2 * C]],
                                    axis=1),
            ea_q_wT=np.ascontiguousarray(ea_wT[:, 0:C]),
            ea_out_wT=wT(inp["ea_out_w"]),
            offaw_wT=wT(offaw_w), ffn_w1T=wT(inp["ffn_w1"]),
            ffn_w2T=wT(inp["ffn_w2"]))
        if triv:
            # fused per-head (value @ out) projections: [384, 384] per head
            wf = [(inp["ms_out_w"][:, h * DH:(h + 1) * DH]
                   @ inp["ms_val_w"][h * DH:(h + 1) * DH, :]).T
                  for h in heads]
            warrs["wf_wT"] = np.ascontiguousarray(
                np.concatenate(wf, axis=1).astype(np.float32))
        else:
            warrs["val_wT_g"] = wT(inp["ms_val_w"][vcols])
            warrs["out_wT"] = wT(inp["ms_out_w"])
        biases = dict(
            sa_in_b=inp["sa_in_b"], sa_out_b=inp["sa_out_b"],
            ea_in_b=inp["ea_in_b"], ea_out_b=inp["ea_out_b"],
            offaw_b=offaw_b, val_b_g=inp["ms_val_b"][vcols],
            out_b=inp["ms_out_b"], ffn_b1=inp["ffn_b1"], ffn_b2=inp["ffn_b2"])
        biasrow = np.concatenate([biases[name].reshape(-1) for name, _ in BIASROW])
        # host-transposed activations: saT [128, 3*64] + eaT [128, 3*128]
        tgtb = inp["tgt"][b]
        extb = inp["extra_memory"][b]
        xT_pack = np.concatenate(
            [tgtb[:, i * 128:(i + 1) * 128].T for i in range(3)]
            + [extb[:, i * 128:(i + 1) * 128].T for i in range(3)], axis=1)
        # (v, q)-partition tables: row p = v*64 + q
        vcol = np.repeat(np.arange(2), Q).astype(np.float32)[:, None]  # [128,1]
        rep2 = lambda row: np.ascontiguousarray(np.broadcast_to(
            row[None, :], (128, NCOL)).astype(np.float32))
        q2parts = dict(
            xb2=(np.tile(np.asarray(inp["reference_points"][b], np.float32),
                         (2, 1))[:, 0:1] * rep2(vr[lcol, 0] * Wl[lcol])
                 - 1.0 + 2.0 ** -9).astype(np.float32),
            yb2=(np.tile(np.asarray(inp["reference_points"][b], np.float32),
                         (2, 1))[:, 1:2] * rep2(vr[lcol, 1] * Hl[lcol])
                 - 1.0 + 2.0 ** -9).astype(np.float32),
            wm12=rep2(Wl[lcol] - 1), wm22=rep2(Wl[lcol] - 2),
            hm22=rep2(Hl[lcol] - 2),
            # y-edge: v=0 -> y0 == -1; v=1 -> y0 == H-1
            etab2=np.where(vcol > 0, rep2(Hl[lcol] - 1), -1.0).astype(np.float32),
            wtab2=rep2(Wl[lcol]),
            base2=(rep2(lsi[lcol].astype(np.float32))
                   + vcol * rep2(Wl[lcol])).astype(np.float32),
            refpts2=np.tile(np.asarray(inp["reference_points"][b], np.float32),
                            (2, 1)),
            sgn2=(2.0 * vcol - 1.0).astype(np.float32),
            vcol2=((1.0 - vcol) + (0.5 - 2.0 ** -9) * (2.0 * vcol - 1.0)
                   ).astype(np.float32),
            wsum2=np.ascontiguousarray(np.broadcast_to(
                offaw_w.sum(1)[None, :], (128, 3 * NCOL)).astype(np.float32)),
            ob2=np.ascontiguousarray(np.broadcast_to(
                offaw_b[None, :], (128, 3 * NCOL)).astype(np.float32)))
        qpk2 = np.concatenate([q2parts[name].reshape(128, n)
                               for name, n in QPK2], axis=1)
        warrs["xT_packW"] = np.ascontiguousarray(xT_pack.astype(np.float32))
        m = dict(
            tgt_in=np.ascontiguousarray(tgtb),
            srcflat_f8=srcpads[b],
            biasrow=np.ascontiguousarray(
                biasrow.reshape(1, -1).astype(ml_dtypes.bfloat16)),
            mpk=mpk,
            qpk2=np.ascontiguousarray(qpk2.astype(np.float32)),
        )
        if not triv:
            qparts = dict(
                ln2_g=repl(inp["ln2_g"]), ln2_b=repl(inp["ln2_b"]),
                lne_g=repl(inp["lne_g"]), lne_b=repl(inp["lne_b"]),
                ln1_g=repl(inp["ln1_g"]), ln1_b=repl(inp["ln1_b"]),
                ln3_g=repl(inp["ln3_g"]), ln3_b=repl(inp["ln3_b"]))
            m["qpk"] = np.ascontiguousarray(np.concatenate(
                [qparts[name].reshape(Q, n) for name, n in QPK],
                axis=1).astype(np.float32))
        for cname, _, items in WCH:
            m[cname] = pack_chunk(items, warrs)
        in_maps.append(m)
    return in_maps


def kernel(**inputs):
    import os
    from concourse.bass_utils import run_bass_kernel_spmd
    triv = _is_triv(inputs) and os.environ.get("KERNEL_FORCE_TRIV") != "0"
    key = ("nc", triv)
    if key not in _CACHE:
        _CACHE[key] = _build(triv=triv)
    nc = _CACHE[key]
    in_maps = make_in_maps(inputs, triv)
    res = run_bass_kernel_spmd(nc, in_maps, core_ids=list(range(N_CORES)))
    out = np.zeros((B, Q, C), np.float32)
    for b in range(B):
        out[b] = res.results[2 * b]["out"]
    return out
